# revision 3
# baseline (speedup 1.0000x reference)
"""DeepseekV3 MLA attention prefill (S=1024, H=128 heads, HID=7168) on 8 TRN2
NeuronCores.

Sharding: tensor-parallel over heads (16 heads/core) for q_b/kv_b/attention/
o_proj; the low-rank input projections (q_a / kv_a) are sequence-sharded
(128 rows/core) and exchanged with one small AllGather of the transposed,
rms-normed activations. Each core emits a partial output projection
(contraction over its own 16 heads); the host sums the 8 partials.

All matmuls run as float32r (full-rate fp32 mode on the PE); softmax and
normalization math stays float32.
"""
import math
import numpy as np

import concourse.bass as bass
import concourse.mybir as mybir
import concourse.bacc as bacc
import concourse.tile as tile
import concourse.bass_utils as bass_utils
from concourse.masks import make_identity
from contextlib import ExitStack

F32 = mybir.dt.float32
F32R = mybir.dt.float32r
AF = mybir.ActivationFunctionType
OP = mybir.AluOpType

N_CORES = 8
S = 1024
HID = 7168
H = 128
HG = H // N_CORES          # 16 heads per core
D_NOPE = 128
D_ROPE = 64
D_Q = D_NOPE + D_ROPE      # 192
D_V = 128
CQ = 1536                  # q lora rank
CKV = 512                  # kv lora rank
CA = CQ + CKV + D_ROPE     # 2112 fused a-proj cols
S_SH = S // N_CORES        # 128 sequence rows per core
CC_A = HID // 128          # 56 contraction chunks for a-proj
NT_A = [(0, 512), (512, 512), (1024, 512), (1536, 512), (2048, 64)]
SCALE = 1.0 / math.sqrt(D_Q)
EPS = 1e-6
G_HEADS = 2                # heads per group
N_GROUPS = HG // G_HEADS   # 8 groups
QT = 2                     # q-tiles of 512 per head
LAST_EXEC_NS = None
LAST_RES = None

_CACHE = {}


def _build_nc():
    nc = bacc.Bacc("TRN2", target_bir_lowering=False, debug=False,
                   num_devices=N_CORES)

    xT = nc.dram_tensor("xT", [HID, S_SH], F32R, kind="ExternalInput")
    wa = nc.dram_tensor("wa", [HID, CA], F32R, kind="ExternalInput")
    qbn = nc.dram_tensor("qbn", [CQ, HG * D_NOPE], F32R, kind="ExternalInput")
    qbp = nc.dram_tensor("qbp", [CQ, HG * D_ROPE], F32R, kind="ExternalInput")
    kvbk = nc.dram_tensor("kvbk", [CKV, HG * D_NOPE], F32R, kind="ExternalInput")
    kvbv = nc.dram_tensor("kvbv", [CKV, HG * D_V], F32R, kind="ExternalInput")
    ow = nc.dram_tensor("ow", [HG * D_V, HID], F32R, kind="ExternalInput")
    cos_s = nc.dram_tensor("cos_s", [S_SH, D_ROPE], F32, kind="ExternalInput")
    sin_sg = nc.dram_tensor("sin_sg", [S_SH, D_ROPE], F32, kind="ExternalInput")
    cos2t = nc.dram_tensor("cos2t", [128, S], F32, kind="ExternalInput")
    sin2tg = nc.dram_tensor("sin2tg", [128, S], F32, kind="ExternalInput")
    masks = nc.dram_tensor("masks", [512, 512], F32, kind="ExternalInput")
    ones_col = nc.dram_tensor("ones_col", [128, 1], F32R, kind="ExternalInput")
    ones_row = nc.dram_tensor("ones_row", [1, 128], F32R, kind="ExternalInput")
    out = nc.dram_tensor("out", [S, HID], F32, kind="ExternalOutput")

    with tile.TileContext(nc) as tc, ExitStack() as top:
        const = top.enter_context(tc.tile_pool(name="const", bufs=1))
        dram = top.enter_context(tc.tile_pool(name="dram", bufs=1, space="DRAM"))
        ps_proj = top.enter_context(tc.tile_pool(name="ps_proj", bufs=2, space="PSUM"))
        ps_sc = top.enter_context(tc.tile_pool(name="ps_sc", bufs=2, space="PSUM"))
        ps_ao = top.enter_context(tc.tile_pool(name="ps_ao", bufs=2, space="PSUM"))
        ps_sm = top.enter_context(tc.tile_pool(name="ps_sm", bufs=2, space="PSUM"))

        # ---- constants in SBUF ----
        ident = const.tile([128, 128], F32, tag="ident")
        make_identity(nc, ident[:])
        masks_sb = const.tile([128, 4, 512], F32, tag="masks")
        for m in range(4):
            nc.sync.dma_start(masks_sb[:, m, :], masks.ap()[m * 128:(m + 1) * 128, :])
        cos_s_sb = const.tile([S_SH, D_ROPE], F32, tag="coss")
        sin_sg_sb = const.tile([S_SH, D_ROPE], F32, tag="sinsg")
        nc.sync.dma_start(cos_s_sb[:], cos_s.ap())
        nc.sync.dma_start(sin_sg_sb[:], sin_sg.ap())
        cos2t_sb = const.tile([128, S], F32, tag="cos2t")
        sin2tg_sb = const.tile([128, S], F32, tag="sin2tg")
        nc.sync.dma_start(cos2t_sb[:], cos2t.ap())
        nc.sync.dma_start(sin2tg_sb[:], sin2tg.ap())
        ones_col_sb = const.tile([128, 1], F32R, tag="onesc")
        ones_row_sb = const.tile([1, 128], F32R, tag="onesr")
        nc.sync.dma_start(ones_col_sb[:], ones_col.ap())
        nc.sync.dma_start(ones_row_sb[:], ones_row.ap())

        agi = dram.tile([CA, S_SH], F32R, tag="agi")
        ago = dram.tile([CA * N_CORES, S_SH], F32R, tag="ago")
        outs_d = dram.tile([HG * D_V, S], F32R, tag="outs")

        # ================= Phase A: fused a-proj + rmsnorm + kpe rope ======
        with ExitStack() as pa:
            sba = pa.enter_context(tc.tile_pool(name="sba", bufs=1))
            sbw = pa.enter_context(tc.tile_pool(name="sbw", bufs=4))
            sbt = pa.enter_context(tc.tile_pool(name="sbt", bufs=2))

            xT_sb = sba.tile([128, CC_A, S_SH], F32R, tag="xT")
            for cc in range(CC_A):
                nc.sync.dma_start(xT_sb[:, cc, :], xT.ap()[cc * 128:(cc + 1) * 128, :])
            acts = sba.tile([S_SH, CA], F32, tag="acts")
            for (d0, dn) in NT_A:
                psum = ps_proj.tile([128, 512], F32, tag="proj")
                for cc in range(CC_A):
                    wt = sbw.tile([128, 512], F32R, tag="wa")
                    nc.sync.dma_start(wt[:, :dn], wa.ap()[cc * 128:(cc + 1) * 128, d0:d0 + dn])
                    nc.tensor.matmul(psum[:, :dn], xT_sb[:, cc, :], wt[:, :dn],
                                     start=(cc == 0), stop=(cc == CC_A - 1))
                nc.scalar.copy(acts[:, d0:d0 + dn], psum[:, :dn])

            # rmsnorm factors for qc (cols 0:1536) and ckv (cols 1536:2048)
            sq = sba.tile([S_SH, CQ + CKV], F32, tag="sq")
            nc.vector.tensor_mul(sq[:], acts[:, 0:CQ + CKV], acts[:, 0:CQ + CKV])
            fq = sbt.tile([S_SH, 1], F32, tag="fq")
            fk = sbt.tile([S_SH, 1], F32, tag="fk")
            nc.vector.reduce_sum(fq[:], sq[:, 0:CQ], axis=mybir.AxisListType.X)
            nc.vector.reduce_sum(fk[:], sq[:, CQ:CQ + CKV], axis=mybir.AxisListType.X)
            nc.vector.tensor_scalar(fq[:], fq[:], 1.0 / CQ, EPS, OP.mult, OP.add)
            nc.vector.tensor_scalar(fk[:], fk[:], 1.0 / CKV, EPS, OP.mult, OP.add)
            nc.vector.reciprocal(fq[:], fq[:])
            nc.vector.reciprocal(fk[:], fk[:])
            nc.scalar.activation(fq[:], fq[:], AF.Sqrt)
            nc.scalar.activation(fk[:], fk[:], AF.Sqrt)
            nc.vector.tensor_scalar_mul(acts[:, 0:CQ], acts[:, 0:CQ], fq[:])
            nc.vector.tensor_scalar_mul(acts[:, CQ:CQ + CKV], acts[:, CQ:CQ + CKV], fk[:])

            # k_pe rope (natural [s, 64] layout), cols 2048:2112
            kp0 = CQ + CKV
            kv1 = sbt.tile([S_SH, D_ROPE], F32, tag="kv1")
            kv2 = sbt.tile([S_SH, D_ROPE], F32, tag="kv2")
            nc.vector.tensor_mul(kv1[:], acts[:, kp0:kp0 + 64], cos_s_sb[:])
            nc.vector.tensor_mul(kv2[:, 0:32], acts[:, kp0 + 32:kp0 + 64], sin_sg_sb[:, 0:32])
            nc.vector.tensor_mul(kv2[:, 32:64], acts[:, kp0:kp0 + 32], sin_sg_sb[:, 32:64])
            nc.vector.tensor_add(acts[:, kp0:kp0 + 64], kv1[:], kv2[:])

            # transpose all 17 chunks -> bounce [2112, 128]
            bT = sba.tile([128, 17 * 128], F32R, tag="bT")
            for t in range(17):
                w = 128 if t < 16 else 64
                pt = ps_proj.tile([128, 512], F32, tag="proj")
                nc.tensor.transpose(pt[:w, 0:128], acts[:, t * 128:t * 128 + w], ident[:])
                nc.scalar.copy(bT[:w, t * 128:(t + 1) * 128], pt[:w, 0:128])
                nc.sync.dma_start(agi[t * 128:t * 128 + w, :], bT[:w, t * 128:(t + 1) * 128])

        nc.gpsimd.collective_compute(
            "AllGather", OP.bypass,
            replica_groups=[list(range(N_CORES))],
            ins=[agi.opt()], outs=[ago.opt()],
        )

        # ================= Phase B: per-head-group projections + attention ==
        with ExitStack() as pb:
            sbg = pb.enter_context(tc.tile_pool(name="sbg", bufs=1))
            sbwq = pb.enter_context(tc.tile_pool(name="sbwq", bufs=1))
            sbh = pb.enter_context(tc.tile_pool(name="sbh", bufs=2))
            sbp = pb.enter_context(tc.tile_pool(name="sbp", bufs=1))
            sbv = pb.enter_context(tc.tile_pool(name="sbv", bufs=2))
            sbs = pb.enter_context(tc.tile_pool(name="sbs", bufs=3))

            # gathered activations, stitched per 512-wide s-tile
            qct = []
            ckv = []
            for st in range(2):
                q_t = sbg.tile([128, CQ // 128, 512], F32R, tag=f"qct{st}")
                k_t = sbg.tile([128, CKV // 128, 512], F32R, tag=f"ckv{st}")
                for r in range(4):
                    core = st * 4 + r
                    base = core * CA
                    for c in range(CQ // 128):
                        nc.sync.dma_start(
                            q_t[:, c, r * 128:(r + 1) * 128],
                            ago[base + c * 128:base + (c + 1) * 128, :])
                    for c in range(CKV // 128):
                        nc.sync.dma_start(
                            k_t[:, c, r * 128:(r + 1) * 128],
                            ago[base + CQ + c * 128:base + CQ + (c + 1) * 128, :])
                qct.append(q_t)
                ckv.append(k_t)
            kpe2 = sbg.tile([128, S], F32R, tag="kpe2")
            for core in range(N_CORES):
                base = core * CA + CQ + CKV
                nc.sync.dma_start(kpe2[0:64, core * 128:(core + 1) * 128],
                                  ago[base:base + 64, :])
                nc.sync.dma_start(kpe2[64:128, core * 128:(core + 1) * 128],
                                  ago[base:base + 64, :])

            for g in range(N_GROUPS):
                h0 = g * G_HEADS
                # --- group weight tiles (one 3-D tile per weight) ---
                qbnw = sbwq.tile([128, CQ // 128, G_HEADS * 128], F32R, tag="qbnw")
                qbpw = sbwq.tile([128, CQ // 128, G_HEADS * 64], F32R, tag="qbpw")
                kvbkw = sbwq.tile([128, CKV // 128, G_HEADS * 128], F32R, tag="kvbkw")
                kvbvw = sbwq.tile([128, CKV // 128, G_HEADS * 128], F32R, tag="kvbvw")
                for c in range(CQ // 128):
                    nc.sync.dma_start(qbnw[:, c, :], qbn.ap()[c * 128:(c + 1) * 128,
                                                              h0 * 128:(h0 + G_HEADS) * 128])
                    nc.sync.dma_start(qbpw[:, c, :], qbp.ap()[c * 128:(c + 1) * 128,
                                                              h0 * 64:(h0 + G_HEADS) * 64])
                for c in range(CKV // 128):
                    nc.sync.dma_start(kvbkw[:, c, :], kvbk.ap()[c * 128:(c + 1) * 128,
                                                                h0 * 128:(h0 + G_HEADS) * 128])
                    nc.sync.dma_start(kvbvw[:, c, :], kvbv.ap()[c * 128:(c + 1) * 128,
                                                                h0 * 128:(h0 + G_HEADS) * 128])

                # --- qT_nope / kT_nope per head; qT_pe pair; v ---
                qTn = []
                kTn = []
                for i in range(G_HEADS):
                    qt_t = sbh.tile([128, S], F32R, tag="qTn")
                    for st in range(2):
                        psum = ps_proj.tile([128, 512], F32, tag="proj")
                        for c in range(CQ // 128):
                            nc.tensor.matmul(psum[:], qbnw[:, c, i * 128:(i + 1) * 128],
                                             qct[st][:, c, :],
                                             start=(c == 0), stop=(c == CQ // 128 - 1))
                        nc.vector.tensor_copy(qt_t[:, st * 512:(st + 1) * 512], psum[:])
                    qTn.append(qt_t)
                    kt_t = sbh.tile([128, S], F32R, tag="kTn")
                    for st in range(2):
                        psum = ps_proj.tile([128, 512], F32, tag="proj")
                        for c in range(CKV // 128):
                            nc.tensor.matmul(psum[:], kvbkw[:, c, i * 128:(i + 1) * 128],
                                             ckv[st][:, c, :],
                                             start=(c == 0), stop=(c == CKV // 128 - 1))
                        nc.vector.tensor_copy(kt_t[:, st * 512:(st + 1) * 512], psum[:])
                    kTn.append(kt_t)

                qp_raw = sbp.tile([128, S], F32, tag="qp_raw")
                for st in range(2):
                    psum = ps_proj.tile([128, 512], F32, tag="proj")
                    for c in range(CQ // 128):
                        nc.tensor.matmul(psum[:], qbpw[:, c, :], qct[st][:, c, :],
                                         start=(c == 0), stop=(c == CQ // 128 - 1))
                    nc.vector.tensor_copy(qp_raw[:, st * 512:(st + 1) * 512], psum[:])
                # rope on the head-pair tile: rows [0:64]=head h0, [64:128]=h0+1
                qTp = sbh.tile([128, S], F32R, tag="qTp")
                rm = sbp.tile([128, S], F32, tag="ropem")
                rs = sbp.tile([128, S], F32, tag="ropes")
                nc.vector.tensor_mul(rm[:], qp_raw[:], cos2t_sb[:])
                # rs = swap32(qp_raw), then multiply by the sign-baked sin table
                for b in range(4):
                    r0 = b * 32
                    r1 = r0 + 32 if b % 2 == 0 else r0 - 32
                    nc.vector.tensor_copy(rs[r0:r0 + 32, :], qp_raw[r1:r1 + 32, :])
                nc.vector.tensor_mul(rs[:], rs[:], sin2tg_sb[:])
                nc.vector.tensor_add(qTp[:], rm[:], rs[:])

                v_g = sbv.tile([128, 8, G_HEADS * 128], F32R, tag="v_g")
                for sc in range(8):
                    st = sc // 4
                    psum = ps_proj.tile([128, 512], F32, tag="proj")
                    nn = G_HEADS * 128
                    for c in range(CKV // 128):
                        nc.tensor.matmul(
                            psum[:, :nn],
                            ckv[st][:, c, (sc % 4) * 128:(sc % 4 + 1) * 128],
                            kvbvw[:, c, :],
                            start=(c == 0), stop=(c == CKV // 128 - 1))
                    nc.vector.tensor_copy(v_g[:, sc, :], psum[:, :nn])

                # --- attention for each head in the group ---
                for i in range(G_HEADS):
                    outT = sbh.tile([128, S], F32R, tag="outT")
                    for qt in range(QT):
                        kmax = 4 * (qt + 1)
                        psum_o = ps_ao.tile([128, 512], F32, tag="o")
                        sums = sbs.tile([128, 512], F32R, tag="sums")
                        for kc in range(kmax):
                            ps = ps_sc.tile([128, 512], F32, tag="s")
                            nc.tensor.matmul(ps[:], kTn[i][:, kc * 128:(kc + 1) * 128],
                                             qTn[i][:, qt * 512:(qt + 1) * 512],
                                             start=True, stop=False)
                            b = i * 64
                            nc.tensor.matmul(ps[:], kpe2[b:b + 64, kc * 128:(kc + 1) * 128],
                                             qTp[b:b + 64, qt * 512:(qt + 1) * 512],
                                             start=False, stop=True)
                            pt = sbs.tile([128, 512], F32R, tag="pt")
                            nc.scalar.activation(pt[:], ps[:], AF.Exp, scale=SCALE)
                            if kc >= 4 * qt:
                                nc.vector.tensor_mul(pt[:], pt[:],
                                                     masks_sb[:, kc - 4 * qt, :])
                            if kc == 0:
                                nc.vector.tensor_copy(sums[:], pt[:])
                            else:
                                nc.vector.tensor_add(sums[:], sums[:], pt[:])
                            nc.tensor.matmul(psum_o[:],
                                             v_g[:, kc, i * 128:(i + 1) * 128], pt[:],
                                             start=(kc == 0), stop=(kc == kmax - 1))
                        pss = ps_sm.tile([128, 512], F32, tag="sm")
                        nc.tensor.matmul(pss[0:1, :], ones_col_sb[:], sums[:],
                                         start=True, stop=True)
                        rec = sbs.tile([1, 512], F32R, tag="rec")
                        with nc.allow_low_precision(reason="softmax recip in f32r"):
                            nc.vector.reciprocal(rec[:], pss[0:1, :])
                        psb = ps_sm.tile([128, 512], F32, tag="sm")
                        nc.tensor.matmul(psb[:], ones_row_sb[:], rec[:],
                                         start=True, stop=True)
                        bsb = sbs.tile([128, 512], F32, tag="bsb")
                        nc.vector.tensor_copy(bsb[:], psb[:])
                        nc.vector.tensor_mul(outT[:, qt * 512:(qt + 1) * 512],
                                             psum_o[:], bsb[:])
                    h_glob = h0 + i
                    nc.sync.dma_start(outs_d[h_glob * 128:(h_glob + 1) * 128, :], outT[:])

        # ================= Phase C: partial output projection ===============
        with ExitStack() as pc:
            sbo = pc.enter_context(tc.tile_pool(name="sbo", bufs=1))
            sbow = pc.enter_context(tc.tile_pool(name="sbow", bufs=18))
            sbos = pc.enter_context(tc.tile_pool(name="sbos", bufs=3))
            sbol = pc.enter_context(tc.tile_pool(name="sbol", bufs=20))
            for nt in range(HID // 512):
                owt = []
                for hc in range(HG):
                    t = sbow.tile([128, 512], F32R, tag="ow")
                    nc.sync.dma_start(t[:], ow.ap()[hc * 128:(hc + 1) * 128,
                                                    nt * 512:(nt + 1) * 512])
                    owt.append(t)
                for st in range(8):
                    lhs = []
                    for hc in range(HG):
                        lt = sbol.tile([128, 128], F32R, tag="ol")
                        nc.sync.dma_start(lt[:], outs_d[hc * 128:(hc + 1) * 128,
                                                        st * 128:(st + 1) * 128])
                        lhs.append(lt)
                    psum = ps_proj.tile([128, 512], F32, tag="proj")
                    for hc in range(HG):
                        nc.tensor.matmul(psum[:], lhs[hc][:], owt[hc][:],
                                         start=(hc == 0), stop=(hc == HG - 1))
                    osb = sbos.tile([128, 512], F32, tag="osb")
                    nc.scalar.copy(osb[:], psum[:])
                    nc.sync.dma_start(out.ap()[st * 128:(st + 1) * 128,
                                               nt * 512:(nt + 1) * 512], osb[:])

    nc.compile()
    return nc


def _host_inputs(hidden_states, position_ids, q_a_weight, q_a_layernorm_weight,
                 q_b_weight, kv_a_weight, kv_a_layernorm_weight, kv_b_weight,
                 o_weight):
    x = np.asarray(hidden_states, np.float32).reshape(S, HID)
    pos = np.asarray(position_ids, np.float64).reshape(S)
    q_a_w = np.asarray(q_a_weight, np.float32)
    q_ln = np.asarray(q_a_layernorm_weight, np.float32)
    q_b_w = np.asarray(q_b_weight, np.float32)
    kv_a_w = np.asarray(kv_a_weight, np.float32)
    kv_ln = np.asarray(kv_a_layernorm_weight, np.float32)
    kv_b_w = np.asarray(kv_b_weight, np.float32)
    o_w = np.asarray(o_weight, np.float32)

    wa = np.concatenate([q_a_w, kv_a_w], axis=1)           # [HID, 2112]
    xT = np.ascontiguousarray(x.T)                          # [HID, S]

    # fold the rms-norm weights into the b-projections
    qb = (q_ln[:, None] * q_b_w).reshape(CQ, H, D_Q)
    kvb = (kv_ln[:, None] * kv_b_w).reshape(CKV, H, D_NOPE + D_V)

    # rope tables
    inv_freq = 1.0 / (10000.0 ** (np.arange(0, D_ROPE, 2, dtype=np.float64) / D_ROPE))
    freqs = pos[:, None] * inv_freq[None, :]                # [S, 32]
    emb = np.concatenate([freqs, freqs], axis=-1)           # [S, 64]
    cos = np.cos(emb).astype(np.float32)
    sin = np.sin(emb).astype(np.float32)
    sin_sg = np.concatenate([-sin[:, :32], sin[:, 32:]], axis=1)  # [S, 64]
    cosT = np.ascontiguousarray(cos.T)                      # [64, S]
    sinT_sg = np.ascontiguousarray(sin_sg.T)                # [64, S]
    cos2t = np.concatenate([cosT, cosT], axis=0)            # [128, S]
    sin2tg = np.concatenate([sinT_sg, sinT_sg], axis=0)     # [128, S]

    # causal masks for the 4 diagonal offsets
    masks = np.zeros((4, 128, 512), np.float32)
    i = np.arange(128)[:, None]
    j = np.arange(512)[None, :]
    for m in range(4):
        masks[m] = ((i + m * 128) <= j).astype(np.float32)
    masks = masks.reshape(512, 512)

    ones_col = np.ones((128, 1), np.float32)
    ones_row = np.ones((1, 128), np.float32)

    in_maps = []
    for c in range(N_CORES):
        hs = slice(c * HG, (c + 1) * HG)
        in_maps.append({
            "xT": np.ascontiguousarray(xT[:, c * S_SH:(c + 1) * S_SH]),
            "wa": wa,
            "qbn": np.ascontiguousarray(qb[:, hs, :D_NOPE].reshape(CQ, HG * D_NOPE)),
            "qbp": np.ascontiguousarray(qb[:, hs, D_NOPE:].reshape(CQ, HG * D_ROPE)),
            "kvbk": np.ascontiguousarray(kvb[:, hs, :D_NOPE].reshape(CKV, HG * D_NOPE)),
            "kvbv": np.ascontiguousarray(kvb[:, hs, D_NOPE:].reshape(CKV, HG * D_V)),
            "ow": np.ascontiguousarray(o_w[c * HG * D_V:(c + 1) * HG * D_V, :]),
            "cos_s": np.ascontiguousarray(cos[c * S_SH:(c + 1) * S_SH, :]),
            "sin_sg": np.ascontiguousarray(sin_sg[c * S_SH:(c + 1) * S_SH, :]),
            "cos2t": cos2t,
            "sin2tg": sin2tg,
            "masks": masks,
            "ones_col": ones_col,
            "ones_row": ones_row,
        })
    return in_maps


def kernel(**inputs):
    global LAST_EXEC_NS, LAST_RES
    trace = bool(inputs.pop("_trace", False))
    in_maps = _host_inputs(**inputs)
    if "nc" not in _CACHE:
        _CACHE["nc"] = _build_nc()
    nc = _CACHE["nc"]
    res = bass_utils.run_bass_kernel_spmd(
        nc, in_maps, core_ids=list(range(N_CORES)), trace=trace)
    LAST_EXEC_NS = res.exec_time_ns
    LAST_RES = res
    total = np.zeros((S, HID), np.float64)
    for c in range(N_CORES):
        total += res.results[c]["out"].astype(np.float64)
    return total.astype(np.float32).reshape(1, 1, S, HID)



# revision 36
# speedup vs baseline: 2.0541x; 2.0541x over previous
"""DeepseekV3 MLA attention prefill (S=1024, H=128, HID=7168) on 8 TRN2 cores.

v2 design (vs v0 baseline at 2.78 ms):
- bf16 weights/activations everywhere (PE rate unchanged, DMA halved,
  rel-err budget ~0.5% << 2e-2 gate).
- Phase A column-sharded: each core computes a [192 q + 64 ckv] column
  slice of the fused a-proj for the FULL sequence, in transposed
  [cols, seq] layout directly (no PE transposes, no stitch-heavy
  AllGather payload). k_pe is computed per-core for its own 128-seq
  slice and roped locally.
- rmsnorm factors from partial sum-squares shipped with the gather;
  scaling applied post-gather via gpsimd broadcast + vector divide.
- Two AllGathers: AG1 (ckv+kpe+sumsq, small) unblocks the whole
  K/V-side projection work, AG2 (q columns) hides behind it.
- Attention at 256-wide q-tiles (25% less wasted causal work vs 512),
  softmax denominator via ones-matmul (reduce+broadcast in one matmul),
  normalization via vector divide. No [1,N] reciprocals.
- outT for all 16 heads kept in SBUF; output projection streams ow
  (bf16) with a deep prefetch pool and accumulates straight from SBUF
  -> dense back-to-back PE work (keeps HAM clock at 2.4 GHz).
"""
import math
import numpy as np
import ml_dtypes

import concourse.bass as bass
import concourse.mybir as mybir
import concourse.bacc as bacc
import concourse.tile as tile
import concourse.bass_isa as bass_isa
import concourse.bass_utils as bass_utils
from contextlib import ExitStack

F32 = mybir.dt.float32
F32R = mybir.dt.float32r
BF16 = mybir.dt.bfloat16
AF = mybir.ActivationFunctionType
OP = mybir.AluOpType
RED = bass_isa.ReduceOp

N_CORES = 8
S = 1024
HID = 7168
H = 128
HG = H // N_CORES          # 16 heads per core
D_NOPE = 128
D_ROPE = 64
D_Q = D_NOPE + D_ROPE      # 192
D_V = 128
CQ = 1536                  # q lora rank
CKV = 512                  # kv lora rank
CC_A = HID // 128          # 56 contraction chunks for a-proj
SCALE = 1.0 / math.sqrt(D_Q)
EPS = 1e-6
N_GROUPS = 8               # head-pair groups per core
QT = 4                     # q-tiles of 256 per head
QW = 256                   # q tile width
LAST_EXEC_NS = None
LAST_RES = None

_CACHE = {}


def _build_nc():
    nc = bacc.Bacc("TRN2", target_bir_lowering=False, debug=False,
                   num_devices=N_CORES)

    xT = nc.dram_tensor("xT", [HID, S], BF16, kind="ExternalInput")
    xkp = nc.dram_tensor("xkp", [HID, 128], BF16, kind="ExternalInput")
    wag1 = nc.dram_tensor("wag1", [HID, 128], BF16, kind="ExternalInput")
    wag2 = nc.dram_tensor("wag2", [HID, 128], BF16, kind="ExternalInput")
    wakp = nc.dram_tensor("wakp", [HID, 64], BF16, kind="ExternalInput")
    qbn = nc.dram_tensor("qbn", [CQ, HG * D_NOPE], BF16, kind="ExternalInput")
    qbp = nc.dram_tensor("qbp", [CQ, HG * D_ROPE], BF16, kind="ExternalInput")
    kvbk = nc.dram_tensor("kvbk", [CKV, HG * D_NOPE], BF16, kind="ExternalInput")
    kvbv = nc.dram_tensor("kvbv", [CKV, HG * D_V], BF16, kind="ExternalInput")
    ow = nc.dram_tensor("ow", [HG * D_V, HID], BF16, kind="ExternalInput")
    costl = nc.dram_tensor("costl", [64, 128], F32, kind="ExternalInput")
    sintl = nc.dram_tensor("sintl", [64, 128], F32, kind="ExternalInput")
    cos2t = nc.dram_tensor("cos2t", [128, S], BF16, kind="ExternalInput")
    sin2tg = nc.dram_tensor("sin2tg", [128, S], BF16, kind="ExternalInput")
    masks2 = nc.dram_tensor("masks2", [128, 2 * QW], BF16, kind="ExternalInput")
    sel = nc.dram_tensor("sel", [128, 4], BF16, kind="ExternalInput")
    onesb = nc.dram_tensor("onesb", [128, 128], F32R, kind="ExternalInput")
    out = nc.dram_tensor("out", [S, HID], F32, kind="ExternalOutput")

    with tile.TileContext(nc) as tc, ExitStack() as top, \
            nc.allow_low_precision(reason="bf16 kernel, 2e-2 rel gate"):
        const = top.enter_context(tc.tile_pool(name="const", bufs=1))
        dram = top.enter_context(tc.tile_pool(name="dram", bufs=1, space="DRAM"))
        ps_pj = top.enter_context(tc.tile_pool(name="ps_pj", bufs=2, space="PSUM"))

        # ---- constants ----
        masks_sb = const.tile([128, 2 * QW], BF16, tag="masks")
        nc.sync.dma_start(masks_sb[:], masks2.ap())
        cos2t_sb = const.tile([128, S], BF16, tag="cos2t")
        sin2tg_sb = const.tile([128, S], BF16, tag="sin2tg")
        nc.sync.dma_start(cos2t_sb[:], cos2t.ap())
        nc.sync.dma_start(sin2tg_sb[:], sin2tg.ap())
        onesb_sb = const.tile([128, 128], F32R, tag="onesb")
        nc.sync.dma_start(onesb_sb[:], onesb.ap())
        sel_sb = const.tile([128, 4], BF16, tag="sel")
        nc.sync.dma_start(sel_sb[:], sel.ap())
        costl_sb = const.tile([64, 128], F32, tag="costl")
        sintl_sb = const.tile([64, 128], F32, tag="sintl")
        nc.sync.dma_start(costl_sb[:], costl.ap())
        nc.sync.dma_start(sintl_sb[:], sintl.ap())

        agi1 = dram.tile([130, S], BF16, tag="agi1")
        ago1 = dram.tile([130 * N_CORES, S], BF16, tag="ago1",
                         addr_space="Shared")
        agi2 = dram.tile([192, S], BF16, tag="agi2")
        ago2 = dram.tile([192 * N_CORES, S], BF16, tag="ago2",
                         addr_space="Shared")

        # ================= Phase A: col-sharded fused a-proj ================
        with ExitStack() as pa:
            sba = pa.enter_context(tc.tile_pool(name="sba", bufs=1))
            psa = pa.enter_context(tc.tile_pool(name="psa", bufs=1, space="PSUM"))

            xt = sba.tile([128, CC_A, S], BF16, tag="xt")
            xkpt = sba.tile([128, CC_A, 128], BF16, tag="xkpt")
            w1t = sba.tile([128, CC_A, 128], BF16, tag="w1t")
            w2t = sba.tile([128, CC_A, 128], BF16, tag="w2t")
            wkt = sba.tile([128, CC_A, 64], BF16, tag="wkt")
            for cc in range(CC_A):
                r = slice(cc * 128, (cc + 1) * 128)
                nc.sync.dma_start(xt[:, cc, :], xT.ap()[r, :])
                nc.sync.dma_start(xkpt[:, cc, :], xkp.ap()[r, :])
                nc.sync.dma_start(w1t[:, cc, :], wag1.ap()[r, :])
                nc.sync.dma_start(w2t[:, cc, :], wag2.ap()[r, :])
                nc.sync.dma_start(wkt[:, cc, :], wakp.ap()[r, :])

            psa1 = [psa.tile([128, 512], F32, tag=f"psa1{st}", name=f"psa1{st}")
                    for st in range(2)]
            psa2 = [psa.tile([128, 512], F32, tag=f"psa2{st}", name=f"psa2{st}")
                    for st in range(2)]
            pskp = psa.tile([64, 128], F32, tag="pskp")
            for cc in range(CC_A):
                st_, sp_ = (cc == 0), (cc == CC_A - 1)
                for st in range(2):
                    mv = xt[:, cc, st * 512:(st + 1) * 512]
                    nc.tensor.matmul(psa1[st][:], w1t[:, cc, :], mv,
                                     start=st_, stop=sp_)
                    nc.tensor.matmul(psa2[st][:], w2t[:, cc, :], mv,
                                     start=st_, stop=sp_)
                nc.tensor.matmul(pskp[:], wkt[:, cc, :], xkpt[:, cc, :],
                                 start=st_, stop=sp_)

            a1b = sba.tile([128, S], BF16, tag="a1b")
            a2b = sba.tile([128, S], BF16, tag="a2b")
            for st in range(2):
                nc.vector.tensor_copy(a1b[:, st * 512:(st + 1) * 512], psa1[st][:])
                nc.vector.tensor_copy(a2b[:, st * 512:(st + 1) * 512], psa2[st][:])

            # k_pe rope on own 128-seq slice (transposed [64, 128] layout)
            kpraw = sba.tile([64, 128], F32, tag="kpraw")
            nc.vector.tensor_copy(kpraw[:], pskp[:])
            kv1 = sba.tile([64, 128], F32, tag="kv1")
            kvs = sba.tile([64, 128], F32, tag="kvs")
            nc.vector.tensor_mul(kv1[:], kpraw[:], costl_sb[:])
            nc.vector.tensor_copy(kvs[0:32, :], kpraw[32:64, :])
            nc.vector.tensor_copy(kvs[32:64, :], kpraw[0:32, :])
            nc.vector.tensor_mul(kvs[:], kvs[:], sintl_sb[:])
            kpb = sba.tile([64, 128], BF16, tag="kpb")
            nc.vector.tensor_add(kpb[:], kv1[:], kvs[:])

            # partial sum-squares: row0 = q-part, row1 = ckv-part
            sq1 = sba.tile([128, S], BF16, tag="sq1")
            sq2 = sba.tile([128, S], BF16, tag="sq2")
            nc.vector.tensor_mul(sq1[:], a1b[:], a1b[:])
            nc.vector.tensor_mul(sq2[:], a2b[:], a2b[:])
            ssb = sba.tile([2, S], BF16, tag="ssb")
            for st in range(2):
                pss = psa.tile([2, 512], F32, tag="pss")
                nc.tensor.matmul(pss[:], sel_sb[:, 0:2],
                                 sq1[:, st * 512:(st + 1) * 512],
                                 start=True, stop=False)
                nc.tensor.matmul(pss[:], sel_sb[:, 2:4],
                                 sq2[:, st * 512:(st + 1) * 512],
                                 start=False, stop=True)
                nc.vector.tensor_copy(ssb[:, st * 512:(st + 1) * 512], pss[:])

            # payloads
            nc.sync.dma_start(agi1[0:64, :], a2b[64:128, :])
            nc.sync.dma_start(agi1[64:128, 0:128], kpb[:])
            nc.sync.dma_start(agi1[128:130, :], ssb[:])
            nc.sync.dma_start(agi2[0:128, :], a1b[:])
            nc.sync.dma_start(agi2[128:192, :], a2b[0:64, :])

        nc.gpsimd.collective_compute(
            "AllGather", OP.bypass,
            replica_groups=[list(range(N_CORES))],
            ins=[agi1.opt()], outs=[ago1.opt()],
        )
        nc.gpsimd.collective_compute(
            "AllGather", OP.bypass,
            replica_groups=[list(range(N_CORES))],
            ins=[agi2.opt()], outs=[ago2.opt()],
        )

        # ================= Phase B =========================================
        with ExitStack() as pb:
            sbo = pb.enter_context(tc.tile_pool(name="sbo", bufs=1))
            outT = sbo.tile([128, HG, S], BF16, tag="outT")
            pbi = pb.enter_context(ExitStack())
            sbq = pbi.enter_context(tc.tile_pool(name="sbq", bufs=1))
            sbvv = pbi.enter_context(tc.tile_pool(name="sbvv", bufs=1))
            sbkw = pbi.enter_context(tc.tile_pool(name="sbkw", bufs=1))
            sbn = pbi.enter_context(tc.tile_pool(name="sbn", bufs=1))
            sbw = pbi.enter_context(tc.tile_pool(name="sbw", bufs=2))
            sbg = pbi.enter_context(tc.tile_pool(name="sbg", bufs=2))
            sbgr = pbi.enter_context(tc.tile_pool(name="sbgr", bufs=1))
            sbs = pbi.enter_context(tc.tile_pool(name="sbs", bufs=2))
            ps_sc = pbi.enter_context(tc.tile_pool(name="ps_sc", bufs=2, space="PSUM"))
            ps_o = pbi.enter_context(tc.tile_pool(name="ps_o", bufs=2, space="PSUM"))
            ps_sb = pbi.enter_context(tc.tile_pool(name="ps_sb", bufs=2, space="PSUM"))

            # --- v-proj weights (no dep on collectives: prefetch early) ---
            kvbvt = sbkw.tile([128, CKV // 128, HG * D_V], BF16, tag="kvbvt")
            for ccc in range(CKV // 128):
                r = slice(ccc * 128, (ccc + 1) * 128)
                nc.sync.dma_start(kvbvt[:, ccc, :], kvbv.ap()[r, :])

            # --- AG1 unpack: ckv chunks, kpe2, sumsq rows ---
            ckv = sbq.tile([128, CKV // 128, S], BF16, tag="ckv")
            for k in range(CKV // 128):
                nc.sync.dma_start(ckv[0:64, k, :],
                                  ago1[130 * (2 * k):130 * (2 * k) + 64, :])
                nc.sync.dma_start(ckv[64:128, k, :],
                                  ago1[130 * (2 * k + 1):130 * (2 * k + 1) + 64, :])
            kpe2 = sbq.tile([128, S], BF16, tag="kpe2")
            for c in range(N_CORES):
                nc.sync.dma_start(kpe2[0:64, c * 128:(c + 1) * 128],
                                  ago1[130 * c + 64:130 * c + 128, 0:128])
            nc.vector.tensor_copy(kpe2[64:128, :], kpe2[0:64, :])
            ssq8 = sbn.tile([8, S], BF16, tag="ssq8")
            ssk8 = sbn.tile([8, S], BF16, tag="ssk8")
            for c in range(N_CORES):
                nc.sync.dma_start(ssq8[c:c + 1, :], ago1[130 * c + 128:130 * c + 129, :])
                nc.sync.dma_start(ssk8[c:c + 1, :], ago1[130 * c + 129:130 * c + 130, :])

            # --- fk: rms denom for ckv; scale ckv in place ---
            prk = sbn.tile([8, S], F32, tag="pr")
            nc.gpsimd.partition_all_reduce(prk[:], ssk8[:], channels=8,
                                           reduce_op=RED.add)
            skv = sbn.tile([1, S], F32, tag="srow")
            nc.vector.tensor_scalar(skv[:], prk[0:1, :], 1.0 / CKV, EPS,
                                    OP.mult, OP.add)
            nc.vector.reciprocal(skv[:], skv[:])
            nc.scalar.activation(skv[:], skv[:], AF.Sqrt)
            skvb = sbn.tile([128, S], F32, tag="sbcast")
            nc.gpsimd.partition_broadcast(skvb[:], skv[:])
            for k in range(CKV // 128):
                nc.vector.tensor_mul(ckv[:, k, :], ckv[:, k, :], skvb[:])

            # --- v-proj for ALL heads (covers AG2 latency) ---
            vv = sbvv.tile([128, 8, HG * D_V], BF16, tag="vv")
            for g in range(N_GROUPS):
                for sc in range(8):
                    pj = ps_pj.tile([128, 512], F32, tag="pj")
                    for ccc in range(CKV // 128):
                        nc.tensor.matmul(
                            pj[:, 0:256],
                            ckv[:, ccc, sc * 128:(sc + 1) * 128],
                            kvbvt[:, ccc, g * 256:(g + 1) * 256],
                            start=(ccc == 0), stop=(ccc == CKV // 128 - 1))
                    nc.vector.tensor_copy(vv[:, sc, g * 256:(g + 1) * 256],
                                          pj[:, 0:256])

            # --- AG2 unpack: q chunks; fq scale ---
            qch = sbq.tile([128, CQ // 128, S], BF16, tag="qch")
            for k in range(8):
                nc.sync.dma_start(qch[:, k, :], ago2[192 * k:192 * k + 128, :])
            for j in range(4):
                nc.sync.dma_start(
                    qch[0:64, 8 + j, :],
                    ago2[192 * (2 * j) + 128:192 * (2 * j) + 192, :])
                nc.sync.dma_start(
                    qch[64:128, 8 + j, :],
                    ago2[192 * (2 * j + 1) + 128:192 * (2 * j + 1) + 192, :])
            prq = sbn.tile([8, S], F32, tag="pr")
            nc.gpsimd.partition_all_reduce(prq[:], ssq8[:], channels=8,
                                           reduce_op=RED.add)
            sqv = sbn.tile([1, S], F32, tag="srow")
            nc.vector.tensor_scalar(sqv[:], prq[0:1, :], 1.0 / CQ, EPS,
                                    OP.mult, OP.add)
            nc.vector.reciprocal(sqv[:], sqv[:])
            nc.scalar.activation(sqv[:], sqv[:], AF.Sqrt)
            sqvb = sbn.tile([128, S], F32, tag="sbcast")
            nc.gpsimd.partition_broadcast(sqvb[:], sqv[:])
            for k in range(CQ // 128):
                nc.vector.tensor_mul(qch[:, k, :], qch[:, k, :], sqvb[:])

            # softmax denominators for all (head, q-tile), reciprocal'd in
            # one batched DVE op at the end (avoids slow [1,N] reciprocals)
            sums_all = sbn.tile([HG, S], F32, tag="sums_all")
            recip_all = sbn.tile([HG, S], F32, tag="recip_all")

            # --- per head-pair group: k/q projections + rope + attention ---
            for g in range(N_GROUPS):
                qbnt = sbw.tile([128, CQ // 128, 256], BF16, tag="qbnt")
                qbpt = sbw.tile([128, CQ // 128, 128], BF16, tag="qbpt")
                kvbkg = sbw.tile([128, CKV // 128, 256], BF16, tag="kvbkg")
                for ccc in range(CQ // 128):
                    r = slice(ccc * 128, (ccc + 1) * 128)
                    nc.sync.dma_start(qbnt[:, ccc, :],
                                      qbn.ap()[r, g * 256:(g + 1) * 256])
                    nc.sync.dma_start(qbpt[:, ccc, :],
                                      qbp.ap()[r, g * 128:(g + 1) * 128])
                for ccc in range(CKV // 128):
                    r = slice(ccc * 128, (ccc + 1) * 128)
                    nc.sync.dma_start(kvbkg[:, ccc, :],
                                      kvbk.ap()[r, g * 256:(g + 1) * 256])

                kTn = sbg.tile([128, 2, S], BF16, tag="kTn")
                for i in range(2):
                    for st in range(2):
                        pj = ps_pj.tile([128, 512], F32, tag="pj")
                        for ccc in range(CKV // 128):
                            nc.tensor.matmul(
                                pj[:], kvbkg[:, ccc, i * 128:(i + 1) * 128],
                                ckv[:, ccc, st * 512:(st + 1) * 512],
                                start=(ccc == 0), stop=(ccc == CKV // 128 - 1))
                        nc.vector.tensor_copy(kTn[:, i, st * 512:(st + 1) * 512],
                                              pj[:])

                qTn2 = sbg.tile([128, 2, S], BF16, tag="qTn2")
                for i in range(2):
                    for st in range(2):
                        pj = ps_pj.tile([128, 512], F32, tag="pj")
                        for ccc in range(CQ // 128):
                            nc.tensor.matmul(
                                pj[:], qbnt[:, ccc, i * 128:(i + 1) * 128],
                                qch[:, ccc, st * 512:(st + 1) * 512],
                                start=(ccc == 0), stop=(ccc == CQ // 128 - 1))
                        nc.vector.tensor_copy(qTn2[:, i, st * 512:(st + 1) * 512],
                                              pj[:])
                qTp = sbg.tile([128, S], BF16, tag="qTp")
                qpr = sbgr.tile([128, S], BF16, tag="qpr")
                for st in range(2):
                    pj = ps_pj.tile([128, 512], F32, tag="pj")
                    for ccc in range(CQ // 128):
                        nc.tensor.matmul(
                            pj[:], qbpt[:, ccc, :],
                            qch[:, ccc, st * 512:(st + 1) * 512],
                            start=(ccc == 0), stop=(ccc == CQ // 128 - 1))
                    nc.vector.tensor_copy(qpr[:, st * 512:(st + 1) * 512], pj[:])
                rm = sbgr.tile([128, S], BF16, tag="rm")
                rs = sbgr.tile([128, S], BF16, tag="rs")
                nc.vector.tensor_mul(rm[:], qpr[:], cos2t_sb[:])
                for b in range(4):
                    r0 = b * 32
                    r1 = r0 + 32 if b % 2 == 0 else r0 - 32
                    nc.vector.tensor_copy(rs[r0:r0 + 32, :], qpr[r1:r1 + 32, :])
                nc.vector.tensor_mul(rs[:], rs[:], sin2tg_sb[:])
                nc.vector.tensor_add(qTp[:], rm[:], rs[:])

                for i in range(2):
                    h = 2 * g + i
                    b = i * 64
                    for qt in range(QT):
                        kmax = 2 * qt + 2
                        po = ps_o.tile([128, QW], F32, tag="po")
                        sums = sbs.tile([128, QW], F32R, tag="sums")
                        for kc in range(kmax):
                            ps = ps_sc.tile([128, QW], F32, tag="ps")
                            nc.tensor.matmul(
                                ps[:], kTn[:, i, kc * 128:(kc + 1) * 128],
                                qTn2[:, i, qt * QW:(qt + 1) * QW],
                                start=True, stop=False)
                            nc.tensor.matmul(
                                ps[:], kpe2[b:b + 64, kc * 128:(kc + 1) * 128],
                                qTp[b:b + 64, qt * QW:(qt + 1) * QW],
                                start=False, stop=True)
                            pt = sbs.tile([128, QW], BF16, tag="pt")
                            nc.scalar.activation(pt[:], ps[:], AF.Exp, scale=SCALE)
                            if kc >= 2 * qt:
                                nc.vector.tensor_mul(
                                    pt[:], pt[:],
                                    masks_sb[:, (kc - 2 * qt) * QW:
                                             (kc - 2 * qt + 1) * QW])
                            if kc == 0:
                                nc.vector.tensor_copy(sums[:], pt[:])
                            else:
                                nc.vector.tensor_add(sums[:], sums[:], pt[:])
                            nc.tensor.matmul(
                                po[:], vv[:, kc, h * 128:(h + 1) * 128], pt[:],
                                start=(kc == 0), stop=(kc == kmax - 1))
                        psb = ps_sb.tile([1, QW], F32, tag="psb")
                        nc.tensor.matmul(psb[:], onesb_sb[:, 0:1], sums[:],
                                         start=True, stop=True)
                        srow = sbs.tile([1, QW], F32, tag="srow")
                        nc.scalar.copy(srow[:], psb[:])
                        nc.sync.dma_start(
                            sums_all[h:h + 1, qt * QW:(qt + 1) * QW], srow[:])
                        nc.vector.tensor_copy(
                            outT[:, h, qt * QW:(qt + 1) * QW], po[:])

            # normalize all heads' outputs: one batched reciprocal, then
            # per-head broadcast + multiply
            nc.vector.reciprocal(recip_all[:], sums_all[:])
            for h in range(HG):
                rrow = sbn.tile([1, S], F32, tag="srow")
                nc.sync.dma_start(rrow[:], recip_all[h:h + 1, :])
                rb = sbn.tile([128, S], F32, tag="sbcast")
                nc.gpsimd.partition_broadcast(rb[:], rrow[:])
                nc.vector.tensor_mul(outT[:, h, :], outT[:, h, :], rb[:])

            pbi.close()  # free all B scratch before phase C pools

            # ================= Phase C: output projection ==================
            with ExitStack() as pc:
                sbow = pc.enter_context(tc.tile_pool(name="sbow", bufs=20))
                sbos = pc.enter_context(tc.tile_pool(name="sbos", bufs=3))
                for nt in range(HID // 512):
                    owt = []
                    for hc in range(HG):
                        t = sbow.tile([128, 512], BF16, tag="ow")
                        nc.sync.dma_start(t[:], ow.ap()[hc * 128:(hc + 1) * 128,
                                                        nt * 512:(nt + 1) * 512])
                        owt.append(t)
                    for st in range(8):
                        pj = ps_pj.tile([128, 512], F32, tag="pj")
                        for hc in range(HG):
                            nc.tensor.matmul(
                                pj[:], outT[:, hc, st * 128:(st + 1) * 128],
                                owt[hc][:], start=(hc == 0), stop=(hc == HG - 1))
                        osb = sbos.tile([128, 512], F32, tag="osb")
                        nc.vector.tensor_copy(osb[:], pj[:])
                        nc.sync.dma_start(out.ap()[st * 128:(st + 1) * 128,
                                                   nt * 512:(nt + 1) * 512], osb[:])

    nc.compile()
    return nc


def _host_inputs(hidden_states, position_ids, q_a_weight, q_a_layernorm_weight,
                 q_b_weight, kv_a_weight, kv_a_layernorm_weight, kv_b_weight,
                 o_weight):
    bf = ml_dtypes.bfloat16
    x = np.asarray(hidden_states, np.float32).reshape(S, HID)
    pos = np.asarray(position_ids, np.float64).reshape(S)
    q_a_w = np.asarray(q_a_weight, np.float32)
    q_ln = np.asarray(q_a_layernorm_weight, np.float32)
    q_b_w = np.asarray(q_b_weight, np.float32)
    kv_a_w = np.asarray(kv_a_weight, np.float32)
    kv_ln = np.asarray(kv_a_layernorm_weight, np.float32)
    kv_b_w = np.asarray(kv_b_weight, np.float32)
    o_w = np.asarray(o_weight, np.float32)

    xT = np.ascontiguousarray(x.T).astype(bf)                 # [HID, S]

    # fold the rms-norm weights into the b-projections
    qb = (q_ln[:, None] * q_b_w).reshape(CQ, H, D_Q)
    kvb = (kv_ln[:, None] * kv_b_w).reshape(CKV, H, D_NOPE + D_V)

    # rope tables
    inv_freq = 1.0 / (10000.0 ** (np.arange(0, D_ROPE, 2, dtype=np.float64) / D_ROPE))
    freqs = pos[:, None] * inv_freq[None, :]                  # [S, 32]
    emb = np.concatenate([freqs, freqs], axis=-1)             # [S, 64]
    cos = np.cos(emb).astype(np.float32)
    sin = np.sin(emb).astype(np.float32)
    sin_sg = np.concatenate([-sin[:, :32], sin[:, 32:]], axis=1)
    cosT = np.ascontiguousarray(cos.T)                        # [64, S]
    sinT_sg = np.ascontiguousarray(sin_sg.T)                  # [64, S]
    cos2t = np.concatenate([cosT, cosT], axis=0).astype(bf)   # [128, S]
    sin2tg = np.concatenate([sinT_sg, sinT_sg], axis=0).astype(bf)

    # causal masks for the two diagonal offsets at 256-wide q tiles
    i = np.arange(128)[:, None]
    j = np.arange(QW)[None, :]
    m0 = (i <= j).astype(np.float32)
    m1 = ((i + 128) <= j).astype(np.float32)
    masks2 = np.concatenate([m0, m1], axis=1)                 # [128, 512]

    # sumsq selectors: cols 0:2 for group1 (all q), 2:4 for group2 (q|ckv)
    sel = np.zeros((128, 4), np.float32)
    sel[:, 0] = 1.0
    sel[0:64, 2] = 1.0
    sel[64:128, 3] = 1.0
    sel = sel.astype(bf)

    onesb = np.ones((128, 128), np.float32)

    qbn_f = qb[:, :, :D_NOPE]                                 # [CQ, H, 128]
    qbp_f = qb[:, :, D_NOPE:]                                 # [CQ, H, 64]
    kvbk_f = kvb[:, :, :D_NOPE]
    kvbv_f = kvb[:, :, D_NOPE:]

    in_maps = []
    for c in range(N_CORES):
        hs = slice(c * HG, (c + 1) * HG)
        # a-proj column slices: g1 = q cols [128c,128c+128);
        # g2 = q cols [1024+64c,+64) ++ ckv cols [64c,+64)
        wag1 = q_a_w[:, 128 * c:128 * (c + 1)]
        wag2 = np.concatenate([
            q_a_w[:, 1024 + 64 * c:1024 + 64 * (c + 1)],
            kv_a_w[:, 64 * c:64 * (c + 1)]], axis=1)
        in_maps.append({
            "xT": xT,
            "xkp": np.ascontiguousarray(xT[:, 128 * c:128 * (c + 1)]),
            "wag1": np.ascontiguousarray(wag1).astype(bf),
            "wag2": np.ascontiguousarray(wag2).astype(bf),
            "wakp": np.ascontiguousarray(kv_a_w[:, CKV:]).astype(bf),
            "qbn": np.ascontiguousarray(
                qbn_f[:, hs, :].reshape(CQ, HG * D_NOPE)).astype(bf),
            "qbp": np.ascontiguousarray(
                qbp_f[:, hs, :].reshape(CQ, HG * D_ROPE)).astype(bf),
            "kvbk": np.ascontiguousarray(
                kvbk_f[:, hs, :].reshape(CKV, HG * D_NOPE)).astype(bf),
            "kvbv": np.ascontiguousarray(
                kvbv_f[:, hs, :].reshape(CKV, HG * D_V)).astype(bf),
            "ow": np.ascontiguousarray(
                o_w[c * HG * D_V:(c + 1) * HG * D_V, :]).astype(bf),
            "costl": np.ascontiguousarray(cosT[:, 128 * c:128 * (c + 1)]),
            "sintl": np.ascontiguousarray(sinT_sg[:, 128 * c:128 * (c + 1)]),
            "cos2t": cos2t,
            "sin2tg": sin2tg,
            "masks2": masks2.astype(bf),
            "sel": sel,
            "onesb": onesb,
        })
    return in_maps


def kernel(**inputs):
    global LAST_EXEC_NS, LAST_RES
    trace = bool(inputs.pop("_trace", False))
    in_maps = _host_inputs(**inputs)
    if "nc" not in _CACHE:
        _CACHE["nc"] = _build_nc()
    nc = _CACHE["nc"]
    res = bass_utils.run_bass_kernel_spmd(
        nc, in_maps, core_ids=list(range(N_CORES)), trace=trace)
    LAST_EXEC_NS = res.exec_time_ns
    LAST_RES = res
    total = np.zeros((S, HID), np.float64)
    for c in range(N_CORES):
        total += res.results[c]["out"].astype(np.float64)
    return total.astype(np.float32).reshape(1, 1, S, HID)


# revision 40
# speedup vs baseline: 2.1422x; 1.0429x over previous
"""DeepseekV3 MLA attention prefill (S=1024, H=128, HID=7168) on 8 TRN2 cores.

v3: tensor-parallel over heads; column-sharded fused a-proj with split
AllGather (kv-side first); bf16 compute; paired-K-block attention
(512-wide exp/mask/sum ops); batched DMAs via host-pre-tiled DRAM
layouts (one dma_start per logical tensor); softmax normalization via
one batched reciprocal + per-head gpsimd broadcast; output projection
streamed from SBUF with deep ow prefetch; partial outputs in bf16,
summed on host.
"""
import math
import numpy as np
import ml_dtypes

import concourse.bass as bass
import concourse.mybir as mybir
import concourse.bacc as bacc
import concourse.tile as tile
import concourse.bass_isa as bass_isa
import concourse.bass_utils as bass_utils
from contextlib import ExitStack

F32 = mybir.dt.float32
F32R = mybir.dt.float32r
BF16 = mybir.dt.bfloat16
AF = mybir.ActivationFunctionType
OP = mybir.AluOpType
RED = bass_isa.ReduceOp

N_CORES = 8
S = 1024
HID = 7168
H = 128
HG = H // N_CORES          # 16 heads per core
D_NOPE = 128
D_ROPE = 64
D_Q = D_NOPE + D_ROPE      # 192
D_V = 128
CQ = 1536                  # q lora rank
CKV = 512                  # kv lora rank
CC_A = HID // 128          # 56 contraction chunks for a-proj
SCALE = 1.0 / math.sqrt(D_Q)
EPS = 1e-6
N_GROUPS = 8               # head-pair groups per core
QT = 4                     # q-tiles of 256 per head
QW = 256                   # q tile width
LAST_EXEC_NS = None
LAST_RES = None

_CACHE = {}


def _build_nc():
    nc = bacc.Bacc("TRN2", target_bir_lowering=False, debug=False,
                   num_devices=N_CORES)

    xT = nc.dram_tensor("xT", [128, CC_A, S], BF16, kind="ExternalInput")
    xkp = nc.dram_tensor("xkp", [128, CC_A, 128], BF16, kind="ExternalInput")
    wag1 = nc.dram_tensor("wag1", [128, CC_A, 128], BF16, kind="ExternalInput")
    wag2 = nc.dram_tensor("wag2", [128, CC_A, 128], BF16, kind="ExternalInput")
    wakp = nc.dram_tensor("wakp", [128, CC_A, 64], BF16, kind="ExternalInput")
    qbn = nc.dram_tensor("qbn", [128, CQ // 128, HG * D_NOPE], BF16,
                         kind="ExternalInput")
    qbp = nc.dram_tensor("qbp", [128, CQ // 128, HG * D_ROPE], BF16,
                         kind="ExternalInput")
    kvbk = nc.dram_tensor("kvbk", [128, CKV // 128, HG * D_NOPE], BF16,
                          kind="ExternalInput")
    kvbv = nc.dram_tensor("kvbv", [128, CKV // 128, HG * D_V], BF16,
                          kind="ExternalInput")
    ow = nc.dram_tensor("ow", [128, HG, HID], BF16, kind="ExternalInput")
    costl = nc.dram_tensor("costl", [64, 128], F32, kind="ExternalInput")
    sintl = nc.dram_tensor("sintl", [64, 128], F32, kind="ExternalInput")
    cos2t = nc.dram_tensor("cos2t", [128, S], BF16, kind="ExternalInput")
    sin2tg = nc.dram_tensor("sin2tg", [128, S], BF16, kind="ExternalInput")
    masks2 = nc.dram_tensor("masks2", [128, 2 * QW], BF16, kind="ExternalInput")
    sel = nc.dram_tensor("sel", [128, 4], BF16, kind="ExternalInput")
    onesb = nc.dram_tensor("onesb", [128, 128], F32R, kind="ExternalInput")
    out = nc.dram_tensor("out", [S, HID], BF16, kind="ExternalOutput")

    with tile.TileContext(nc) as tc, ExitStack() as top, \
            nc.allow_low_precision(reason="bf16 kernel, 2e-2 rel gate"):
        const = top.enter_context(tc.tile_pool(name="const", bufs=1))
        dram = top.enter_context(tc.tile_pool(name="dram", bufs=1, space="DRAM"))
        ps_pj = top.enter_context(tc.tile_pool(name="ps_pj", bufs=2, space="PSUM"))

        # ---- constants ----
        masks_sb = const.tile([128, 2 * QW], BF16, tag="masks")
        nc.sync.dma_start(masks_sb[:], masks2.ap())
        cos2t_sb = const.tile([128, S], BF16, tag="cos2t")
        sin2tg_sb = const.tile([128, S], BF16, tag="sin2tg")
        nc.sync.dma_start(cos2t_sb[:], cos2t.ap())
        nc.sync.dma_start(sin2tg_sb[:], sin2tg.ap())
        onesb_sb = const.tile([128, 128], F32R, tag="onesb")
        nc.sync.dma_start(onesb_sb[:], onesb.ap())
        sel_sb = const.tile([128, 4], BF16, tag="sel")
        nc.sync.dma_start(sel_sb[:], sel.ap())
        costl_sb = const.tile([64, 128], F32, tag="costl")
        sintl_sb = const.tile([64, 128], F32, tag="sintl")
        nc.sync.dma_start(costl_sb[:], costl.ap())
        nc.sync.dma_start(sintl_sb[:], sintl.ap())

        agi1 = dram.tile([130, S], BF16, tag="agi1")
        ago1 = dram.tile([130 * N_CORES, S], BF16, tag="ago1",
                         addr_space="Shared")
        agi2 = dram.tile([192, S], BF16, tag="agi2")
        ago2 = dram.tile([192 * N_CORES, S], BF16, tag="ago2",
                         addr_space="Shared")

        # ================= Phase A: col-sharded fused a-proj ================
        with ExitStack() as pa:
            sba = pa.enter_context(tc.tile_pool(name="sba", bufs=1))
            psa = pa.enter_context(tc.tile_pool(name="psa", bufs=1, space="PSUM"))

            xt = sba.tile([128, CC_A, S], BF16, tag="xt")
            xkpt = sba.tile([128, CC_A, 128], BF16, tag="xkpt")
            w1t = sba.tile([128, CC_A, 128], BF16, tag="w1t")
            w2t = sba.tile([128, CC_A, 128], BF16, tag="w2t")
            wkt = sba.tile([128, CC_A, 64], BF16, tag="wkt")
            nc.sync.dma_start(xt[:], xT.ap())
            nc.sync.dma_start(xkpt[:], xkp.ap())
            nc.sync.dma_start(w1t[:], wag1.ap())
            nc.sync.dma_start(w2t[:], wag2.ap())
            nc.sync.dma_start(wkt[:], wakp.ap())

            psa1 = [psa.tile([128, 512], F32, tag=f"psa1{st}", name=f"psa1{st}")
                    for st in range(2)]
            psa2 = [psa.tile([128, 512], F32, tag=f"psa2{st}", name=f"psa2{st}")
                    for st in range(2)]
            pskp = psa.tile([64, 128], F32, tag="pskp")
            for cc in range(CC_A):
                st_, sp_ = (cc == 0), (cc == CC_A - 1)
                for st in range(2):
                    mv = xt[:, cc, st * 512:(st + 1) * 512]
                    nc.tensor.matmul(psa1[st][:], w1t[:, cc, :], mv,
                                     start=st_, stop=sp_)
                    nc.tensor.matmul(psa2[st][:], w2t[:, cc, :], mv,
                                     start=st_, stop=sp_)
                nc.tensor.matmul(pskp[:], wkt[:, cc, :], xkpt[:, cc, :],
                                 start=st_, stop=sp_)

            a1b = sba.tile([128, S], BF16, tag="a1b")
            a2b = sba.tile([128, S], BF16, tag="a2b")
            for st in range(2):
                nc.vector.tensor_copy(a1b[:, st * 512:(st + 1) * 512], psa1[st][:])
                nc.vector.tensor_copy(a2b[:, st * 512:(st + 1) * 512], psa2[st][:])

            # k_pe rope on own 128-seq slice (transposed [64, 128] layout)
            kpraw = sba.tile([64, 128], F32, tag="kpraw")
            nc.vector.tensor_copy(kpraw[:], pskp[:])
            kv1 = sba.tile([64, 128], F32, tag="kv1")
            kvs = sba.tile([64, 128], F32, tag="kvs")
            nc.vector.tensor_mul(kv1[:], kpraw[:], costl_sb[:])
            nc.vector.tensor_copy(kvs[0:32, :], kpraw[32:64, :])
            nc.vector.tensor_copy(kvs[32:64, :], kpraw[0:32, :])
            nc.vector.tensor_mul(kvs[:], kvs[:], sintl_sb[:])
            kpb = sba.tile([64, 128], BF16, tag="kpb")
            nc.vector.tensor_add(kpb[:], kv1[:], kvs[:])

            # partial mean-squares (sel is pre-scaled by 1/CQ, 1/CKV):
            # row0 = q-part, row1 = ckv-part
            sq1 = sba.tile([128, S], BF16, tag="sq1")
            sq2 = sba.tile([128, S], BF16, tag="sq2")
            nc.vector.tensor_mul(sq1[:], a1b[:], a1b[:])
            nc.vector.tensor_mul(sq2[:], a2b[:], a2b[:])
            ssb = sba.tile([2, S], BF16, tag="ssb")
            for st in range(2):
                pss = psa.tile([2, 512], F32, tag="pss")
                nc.tensor.matmul(pss[:], sel_sb[:, 0:2],
                                 sq1[:, st * 512:(st + 1) * 512],
                                 start=True, stop=False)
                nc.tensor.matmul(pss[:], sel_sb[:, 2:4],
                                 sq2[:, st * 512:(st + 1) * 512],
                                 start=False, stop=True)
                nc.vector.tensor_copy(ssb[:, st * 512:(st + 1) * 512], pss[:])

            # payloads
            nc.sync.dma_start(agi1[0:64, :], a2b[64:128, :])
            nc.sync.dma_start(agi1[64:128, 0:128], kpb[:])
            nc.sync.dma_start(agi1[128:130, :], ssb[:])
            nc.sync.dma_start(agi2[0:128, :], a1b[:])
            nc.sync.dma_start(agi2[128:192, :], a2b[0:64, :])

        nc.gpsimd.collective_compute(
            "AllGather", OP.bypass,
            replica_groups=[list(range(N_CORES))],
            ins=[agi1.opt()], outs=[ago1.opt()],
        )
        nc.gpsimd.collective_compute(
            "AllGather", OP.bypass,
            replica_groups=[list(range(N_CORES))],
            ins=[agi2.opt()], outs=[ago2.opt()],
        )

        # ================= Phase B =========================================
        with ExitStack() as pb:
            sbo = pb.enter_context(tc.tile_pool(name="sbo", bufs=1))
            outT = sbo.tile([128, HG, S], BF16, tag="outT")
            pbi = pb.enter_context(ExitStack())
            sbq = pbi.enter_context(tc.tile_pool(name="sbq", bufs=1))
            sbvv = pbi.enter_context(tc.tile_pool(name="sbvv", bufs=1))
            sbkw = pbi.enter_context(tc.tile_pool(name="sbkw", bufs=1))
            sbn = pbi.enter_context(tc.tile_pool(name="sbn", bufs=1))
            sbw = pbi.enter_context(tc.tile_pool(name="sbw", bufs=2))
            sbg = pbi.enter_context(tc.tile_pool(name="sbg", bufs=2))
            sbgr = pbi.enter_context(tc.tile_pool(name="sbgr", bufs=1))
            sbs = pbi.enter_context(tc.tile_pool(name="sbs", bufs=2))
            ps_sc = pbi.enter_context(tc.tile_pool(name="ps_sc", bufs=2, space="PSUM"))
            ps_o = pbi.enter_context(tc.tile_pool(name="ps_o", bufs=2, space="PSUM"))
            ps_sb = pbi.enter_context(tc.tile_pool(name="ps_sb", bufs=2, space="PSUM"))

            # --- v-proj weights (no dep on collectives: prefetch early) ---
            kvbvt = sbkw.tile([128, CKV // 128, HG * D_V], BF16, tag="kvbvt")
            nc.sync.dma_start(kvbvt[:], kvbv.ap())

            ago1v = ago1.rearrange("(c r) s -> r c s", r=130)
            ago1p = ago1.rearrange("(cp two r) s -> two r cp s", two=2, r=130)
            ago2v = ago2.rearrange("(c r) s -> r c s", r=192)
            ago2p = ago2.rearrange("(cp two r) s -> two r cp s", two=2, r=192)

            # --- AG1 unpack: ckv chunks, kpe2, mean-square rows ---
            ckv = sbq.tile([128, CKV // 128, S], BF16, tag="ckv")
            nc.sync.dma_start(ckv[0:64, :, :], ago1p[0, 0:64, :, :])
            nc.sync.dma_start(ckv[64:128, :, :], ago1p[1, 0:64, :, :])
            kpe2 = sbq.tile([128, S], BF16, tag="kpe2")
            nc.sync.dma_start(
                kpe2[0:64, :].rearrange("r (c s) -> r c s", c=8),
                ago1v[64:128, :, 0:128])
            nc.vector.tensor_copy(kpe2[64:128, :], kpe2[0:64, :])
            ssq8 = sbn.tile([8, S], BF16, tag="ssq8")
            ssk8 = sbn.tile([8, S], BF16, tag="ssk8")
            nc.sync.dma_start(ssq8[:], ago1v[128, :, :])
            nc.sync.dma_start(ssk8[:], ago1v[129, :, :])

            # --- rsqrt(mean+eps) for both norms, batched ---
            prq = sbn.tile([8, S], F32, tag="prq")
            prk = sbn.tile([8, S], F32, tag="prk")
            nc.gpsimd.partition_all_reduce(prq[:], ssq8[:], channels=8,
                                           reduce_op=RED.add)
            nc.gpsimd.partition_all_reduce(prk[:], ssk8[:], channels=8,
                                           reduce_op=RED.add)
            nrm2 = sbn.tile([2, S], F32, tag="rows")
            nc.sync.dma_start(nrm2[0:1, :], prq[0:1, :])
            nc.sync.dma_start(nrm2[1:2, :], prk[0:1, :])
            nc.vector.tensor_scalar_add(nrm2[:], nrm2[:], EPS)
            nc.vector.reciprocal(nrm2[:], nrm2[:])
            nc.scalar.activation(nrm2[:], nrm2[:], AF.Sqrt)
            fkrow = sbn.tile([1, S], F32, tag="rows2")
            nc.sync.dma_start(fkrow[:], nrm2[1:2, :])
            fkb = sbn.tile([128, S], F32, tag="bcast")
            nc.gpsimd.partition_broadcast(fkb[:], fkrow[:])
            for k in range(CKV // 128):
                nc.vector.tensor_mul(ckv[:, k, :], ckv[:, k, :], fkb[:])

            # --- v-proj for ALL heads (covers AG2 latency) ---
            vv = sbvv.tile([128, 8, HG * D_V], BF16, tag="vv")
            for g in range(N_GROUPS):
                for sc in range(8):
                    pj = ps_pj.tile([128, 512], F32, tag="pj")
                    for ccc in range(CKV // 128):
                        nc.tensor.matmul(
                            pj[:, 0:256],
                            ckv[:, ccc, sc * 128:(sc + 1) * 128],
                            kvbvt[:, ccc, g * 256:(g + 1) * 256],
                            start=(ccc == 0), stop=(ccc == CKV // 128 - 1))
                    nc.vector.tensor_copy(vv[:, sc, g * 256:(g + 1) * 256],
                                          pj[:, 0:256])

            # --- AG2 unpack: q chunks; fq scale ---
            qch = sbq.tile([128, CQ // 128, S], BF16, tag="qch")
            nc.sync.dma_start(qch[:, 0:8, :], ago2v[0:128, :, :])
            nc.sync.dma_start(qch[0:64, 8:12, :], ago2p[0, 128:192, :, :])
            nc.sync.dma_start(qch[64:128, 8:12, :], ago2p[1, 128:192, :, :])
            fqb = sbn.tile([128, S], F32, tag="bcast")
            nc.gpsimd.partition_broadcast(fqb[:], nrm2[0:1, :])
            for k in range(CQ // 128):
                nc.vector.tensor_mul(qch[:, k, :], qch[:, k, :], fqb[:])

            # softmax denominators for all (head, q-tile); one batched
            # reciprocal at the end (avoids slow [1,N] reciprocals)
            sums_all = sbn.tile([HG, S], F32, tag="sums_all")
            recip_all = sbn.tile([HG, S], F32, tag="recip_all")

            # --- per head-pair group: k/q projections + rope + attention ---
            for g in range(N_GROUPS):
                qbnt = sbw.tile([128, CQ // 128, 256], BF16, tag="qbnt")
                qbpt = sbw.tile([128, CQ // 128, 128], BF16, tag="qbpt")
                kvbkg = sbw.tile([128, CKV // 128, 256], BF16, tag="kvbkg")
                nc.sync.dma_start(qbnt[:], qbn.ap()[:, :, g * 256:(g + 1) * 256])
                nc.sync.dma_start(qbpt[:], qbp.ap()[:, :, g * 128:(g + 1) * 128])
                nc.sync.dma_start(kvbkg[:], kvbk.ap()[:, :, g * 256:(g + 1) * 256])

                kTn = sbg.tile([128, 2, S], BF16, tag="kTn")
                for i in range(2):
                    for st in range(2):
                        pj = ps_pj.tile([128, 512], F32, tag="pj")
                        for ccc in range(CKV // 128):
                            nc.tensor.matmul(
                                pj[:], kvbkg[:, ccc, i * 128:(i + 1) * 128],
                                ckv[:, ccc, st * 512:(st + 1) * 512],
                                start=(ccc == 0), stop=(ccc == CKV // 128 - 1))
                        nc.vector.tensor_copy(kTn[:, i, st * 512:(st + 1) * 512],
                                              pj[:])
                qTn2 = sbg.tile([128, 2, S], BF16, tag="qTn2")
                for i in range(2):
                    for st in range(2):
                        pj = ps_pj.tile([128, 512], F32, tag="pj")
                        for ccc in range(CQ // 128):
                            nc.tensor.matmul(
                                pj[:], qbnt[:, ccc, i * 128:(i + 1) * 128],
                                qch[:, ccc, st * 512:(st + 1) * 512],
                                start=(ccc == 0), stop=(ccc == CQ // 128 - 1))
                        nc.vector.tensor_copy(qTn2[:, i, st * 512:(st + 1) * 512],
                                              pj[:])
                qTp = sbg.tile([128, S], BF16, tag="qTp")
                qpr = sbgr.tile([128, S], BF16, tag="qpr")
                for st in range(2):
                    pj = ps_pj.tile([128, 512], F32, tag="pj")
                    for ccc in range(CQ // 128):
                        nc.tensor.matmul(
                            pj[:], qbpt[:, ccc, :],
                            qch[:, ccc, st * 512:(st + 1) * 512],
                            start=(ccc == 0), stop=(ccc == CQ // 128 - 1))
                    nc.vector.tensor_copy(qpr[:, st * 512:(st + 1) * 512], pj[:])
                rs = sbgr.tile([128, S], BF16, tag="rs")
                nc.vector.tensor_mul(qTp[:], qpr[:], cos2t_sb[:])
                for b in range(4):
                    r0 = b * 32
                    r1 = r0 + 32 if b % 2 == 0 else r0 - 32
                    nc.vector.tensor_copy(rs[r0:r0 + 32, :], qpr[r1:r1 + 32, :])
                nc.vector.tensor_mul(rs[:], rs[:], sin2tg_sb[:])
                nc.vector.tensor_add(qTp[:], qTp[:], rs[:])

                for i in range(2):
                    h = 2 * g + i
                    b = i * 64
                    for qt in range(QT):
                        po = ps_o.tile([128, QW], F32, tag="po")
                        sums = sbs.tile([128, 2 * QW], F32R, tag="sums")
                        for j in range(qt + 1):
                            ps = ps_sc.tile([128, 2 * QW], F32, tag="ps")
                            for half in range(2):
                                kc = 2 * j + half
                                hs = slice(half * QW, (half + 1) * QW)
                                nc.tensor.matmul(
                                    ps[:, hs],
                                    kTn[:, i, kc * 128:(kc + 1) * 128],
                                    qTn2[:, i, qt * QW:(qt + 1) * QW],
                                    start=True, stop=False)
                                nc.tensor.matmul(
                                    ps[:, hs],
                                    kpe2[b:b + 64, kc * 128:(kc + 1) * 128],
                                    qTp[b:b + 64, qt * QW:(qt + 1) * QW],
                                    start=False, stop=True)
                            pt = sbs.tile([128, 2 * QW], BF16, tag="pt")
                            nc.scalar.activation(pt[:], ps[:], AF.Exp, scale=SCALE)
                            if j == qt:
                                nc.vector.tensor_mul(pt[:], pt[:], masks_sb[:])
                            if j == 0:
                                nc.vector.tensor_copy(sums[:], pt[:])
                            else:
                                nc.vector.tensor_add(sums[:], sums[:], pt[:])
                            nc.tensor.matmul(
                                po[:], vv[:, 2 * j, h * 128:(h + 1) * 128],
                                pt[:, 0:QW], start=(j == 0), stop=False)
                            nc.tensor.matmul(
                                po[:], vv[:, 2 * j + 1, h * 128:(h + 1) * 128],
                                pt[:, QW:2 * QW], start=False, stop=(j == qt))
                        psb = ps_sb.tile([1, QW], F32, tag="psb")
                        nc.tensor.matmul(psb[:], onesb_sb[:, 0:1], sums[:, 0:QW],
                                         start=True, stop=False)
                        nc.tensor.matmul(psb[:], onesb_sb[:, 0:1], sums[:, QW:],
                                         start=False, stop=True)
                        srow = sbs.tile([1, QW], F32, tag="srow")
                        nc.scalar.copy(srow[:], psb[:])
                        nc.sync.dma_start(
                            sums_all[h:h + 1, qt * QW:(qt + 1) * QW], srow[:])
                        nc.vector.tensor_copy(
                            outT[:, h, qt * QW:(qt + 1) * QW], po[:])

            # normalize all heads' outputs: one batched reciprocal, then
            # per-head broadcast + multiply
            nc.vector.reciprocal(recip_all[:], sums_all[:])
            for h in range(HG):
                rrow = sbn.tile([1, S], F32, tag="rows")
                nc.sync.dma_start(rrow[:], recip_all[h:h + 1, :])
                rb = sbn.tile([128, S], F32, tag="bcast")
                nc.gpsimd.partition_broadcast(rb[:], rrow[:])
                nc.vector.tensor_mul(outT[:, h, :], outT[:, h, :], rb[:])

            pbi.close()  # free all B scratch before phase C pools

            # ================= Phase C: output projection ==================
            with ExitStack() as pc:
                sbow = pc.enter_context(tc.tile_pool(name="sbow", bufs=3))
                sbos = pc.enter_context(tc.tile_pool(name="sbos", bufs=2))
                outv = out.ap().rearrange("(st p) n -> p st n", p=128)
                for nt in range(HID // 512):
                    owt = sbow.tile([128, HG, 512], BF16, tag="ow")
                    nc.sync.dma_start(owt[:], ow.ap()[:, :, nt * 512:(nt + 1) * 512])
                    osb = sbos.tile([128, 8, 512], BF16, tag="osb")
                    for st in range(8):
                        pj = ps_pj.tile([128, 512], F32, tag="pj")
                        for hc in range(HG):
                            nc.tensor.matmul(
                                pj[:], outT[:, hc, st * 128:(st + 1) * 128],
                                owt[:, hc, :], start=(hc == 0), stop=(hc == HG - 1))
                        nc.vector.tensor_copy(osb[:, st, :], pj[:])
                    nc.sync.dma_start(outv[:, :, nt * 512:(nt + 1) * 512], osb[:])

    nc.compile()
    return nc


def _tile_rows(a, p=128):
    """[R, N] -> [p, R//p, N] with row r = (cc*p + part)."""
    r, n = a.shape
    return np.ascontiguousarray(a.reshape(r // p, p, n).transpose(1, 0, 2))


def _host_inputs(hidden_states, position_ids, q_a_weight, q_a_layernorm_weight,
                 q_b_weight, kv_a_weight, kv_a_layernorm_weight, kv_b_weight,
                 o_weight):
    bf = ml_dtypes.bfloat16
    x = np.asarray(hidden_states, np.float32).reshape(S, HID)
    pos = np.asarray(position_ids, np.float64).reshape(S)
    q_a_w = np.asarray(q_a_weight, np.float32)
    q_ln = np.asarray(q_a_layernorm_weight, np.float32)
    q_b_w = np.asarray(q_b_weight, np.float32)
    kv_a_w = np.asarray(kv_a_weight, np.float32)
    kv_ln = np.asarray(kv_a_layernorm_weight, np.float32)
    kv_b_w = np.asarray(kv_b_weight, np.float32)
    o_w = np.asarray(o_weight, np.float32)

    xT = np.ascontiguousarray(x.T).astype(bf)                 # [HID, S]
    xT_t = _tile_rows(xT)                                     # [128, 56, S]

    qb = (q_ln[:, None] * q_b_w).reshape(CQ, H, D_Q)
    kvb = (kv_ln[:, None] * kv_b_w).reshape(CKV, H, D_NOPE + D_V)

    inv_freq = 1.0 / (10000.0 ** (np.arange(0, D_ROPE, 2, dtype=np.float64) / D_ROPE))
    freqs = pos[:, None] * inv_freq[None, :]
    emb = np.concatenate([freqs, freqs], axis=-1)
    cos = np.cos(emb).astype(np.float32)
    sin = np.sin(emb).astype(np.float32)
    sin_sg = np.concatenate([-sin[:, :32], sin[:, 32:]], axis=1)
    cosT = np.ascontiguousarray(cos.T)                        # [64, S]
    sinT_sg = np.ascontiguousarray(sin_sg.T)
    cos2t = np.concatenate([cosT, cosT], axis=0).astype(bf)
    sin2tg = np.concatenate([sinT_sg, sinT_sg], axis=0).astype(bf)

    i = np.arange(128)[:, None]
    j = np.arange(QW)[None, :]
    m0 = (i <= j).astype(np.float32)
    m1 = ((i + 128) <= j).astype(np.float32)
    masks2 = np.concatenate([m0, m1], axis=1).astype(bf)

    # mean-square selectors, pre-scaled by 1/CQ (q cols) and 1/CKV (ckv)
    sel = np.zeros((128, 4), np.float32)
    sel[:, 0] = 1.0 / CQ
    sel[0:64, 2] = 1.0 / CQ
    sel[64:128, 3] = 1.0 / CKV
    sel = sel.astype(bf)

    onesb = np.ones((128, 128), np.float32)

    qbn_f = qb[:, :, :D_NOPE]
    qbp_f = qb[:, :, D_NOPE:]
    kvbk_f = kvb[:, :, :D_NOPE]
    kvbv_f = kvb[:, :, D_NOPE:]

    in_maps = []
    for c in range(N_CORES):
        hs = slice(c * HG, (c + 1) * HG)
        wag1 = q_a_w[:, 128 * c:128 * (c + 1)]
        wag2 = np.concatenate([
            q_a_w[:, 1024 + 64 * c:1024 + 64 * (c + 1)],
            kv_a_w[:, 64 * c:64 * (c + 1)]], axis=1)
        ow_c = o_w[c * HG * D_V:(c + 1) * HG * D_V, :].astype(bf)  # [2048, HID]
        in_maps.append({
            "xT": xT_t,
            "xkp": _tile_rows(np.ascontiguousarray(
                xT[:, 128 * c:128 * (c + 1)])),
            "wag1": _tile_rows(np.ascontiguousarray(wag1).astype(bf)),
            "wag2": _tile_rows(np.ascontiguousarray(wag2).astype(bf)),
            "wakp": _tile_rows(np.ascontiguousarray(kv_a_w[:, CKV:]).astype(bf)),
            "qbn": _tile_rows(np.ascontiguousarray(
                qbn_f[:, hs, :].reshape(CQ, HG * D_NOPE)).astype(bf)),
            "qbp": _tile_rows(np.ascontiguousarray(
                qbp_f[:, hs, :].reshape(CQ, HG * D_ROPE)).astype(bf)),
            "kvbk": _tile_rows(np.ascontiguousarray(
                kvbk_f[:, hs, :].reshape(CKV, HG * D_NOPE)).astype(bf)),
            "kvbv": _tile_rows(np.ascontiguousarray(
                kvbv_f[:, hs, :].reshape(CKV, HG * D_V)).astype(bf)),
            "ow": _tile_rows(ow_c),                           # [128, 16, HID]
            "costl": np.ascontiguousarray(cosT[:, 128 * c:128 * (c + 1)]),
            "sintl": np.ascontiguousarray(sinT_sg[:, 128 * c:128 * (c + 1)]),
            "cos2t": cos2t,
            "sin2tg": sin2tg,
            "masks2": masks2,
            "sel": sel,
            "onesb": onesb,
        })
    return in_maps


def kernel(**inputs):
    global LAST_EXEC_NS, LAST_RES
    trace = bool(inputs.pop("_trace", False))
    in_maps = _host_inputs(**inputs)
    if "nc" not in _CACHE:
        _CACHE["nc"] = _build_nc()
    nc = _CACHE["nc"]
    res = bass_utils.run_bass_kernel_spmd(
        nc, in_maps, core_ids=list(range(N_CORES)), trace=trace)
    LAST_EXEC_NS = res.exec_time_ns
    LAST_RES = res
    total = np.zeros((S, HID), np.float64)
    for c in range(N_CORES):
        total += res.results[c]["out"].astype(np.float64)
    return total.astype(np.float32).reshape(1, 1, S, HID)


# revision 49
# speedup vs baseline: 2.2292x; 1.0406x over previous
"""DeepseekV3 MLA attention prefill (S=1024, H=128, HID=7168) on 8 TRN2 cores.

v3: tensor-parallel over heads; column-sharded fused a-proj with split
AllGather (kv-side first); bf16 compute; paired-K-block attention
(512-wide exp/mask/sum ops); batched DMAs via host-pre-tiled DRAM
layouts (one dma_start per logical tensor); softmax normalization via
one batched reciprocal + per-head gpsimd broadcast; output projection
streamed from SBUF with deep ow prefetch; partial outputs in bf16,
summed on host.
"""
import math
import numpy as np
import ml_dtypes

import concourse.bass as bass
import concourse.mybir as mybir
import concourse.bacc as bacc
import concourse.tile as tile
import concourse.bass_isa as bass_isa
import concourse.bass_utils as bass_utils
from concourse.masks import make_identity
from contextlib import ExitStack

F32 = mybir.dt.float32
F32R = mybir.dt.float32r
BF16 = mybir.dt.bfloat16
AF = mybir.ActivationFunctionType
OP = mybir.AluOpType
RED = bass_isa.ReduceOp

N_CORES = 8
S = 1024
HID = 7168
H = 128
HG = H // N_CORES          # 16 heads per core
D_NOPE = 128
D_ROPE = 64
D_Q = D_NOPE + D_ROPE      # 192
D_V = 128
CQ = 1536                  # q lora rank
CKV = 512                  # kv lora rank
CC_A = HID // 128          # 56 contraction chunks for a-proj
SCALE = 1.0 / math.sqrt(D_Q)
EPS = 1e-6
N_GROUPS = 8               # head-pair groups per core
QT = 4                     # q-tiles of 256 per head
QW = 256                   # q tile width
LAST_EXEC_NS = None
LAST_RES = None

_CACHE = {}


def _build_nc():
    nc = bacc.Bacc("TRN2", target_bir_lowering=False, debug=False,
                   num_devices=N_CORES)

    xT = nc.dram_tensor("xT", [128, CC_A, S], BF16, kind="ExternalInput")
    xkp = nc.dram_tensor("xkp", [128, CC_A, 128], BF16, kind="ExternalInput")
    wag1 = nc.dram_tensor("wag1", [128, CC_A, 128], BF16, kind="ExternalInput")
    wag2 = nc.dram_tensor("wag2", [128, CC_A, 128], BF16, kind="ExternalInput")
    wakp = nc.dram_tensor("wakp", [128, CC_A, 64], BF16, kind="ExternalInput")
    qbn = nc.dram_tensor("qbn", [128, CQ // 128, HG * D_NOPE], BF16,
                         kind="ExternalInput")
    qbp = nc.dram_tensor("qbp", [128, CQ // 128, HG * D_ROPE], BF16,
                         kind="ExternalInput")
    kvbk = nc.dram_tensor("kvbk", [128, CKV // 128, HG * D_NOPE], BF16,
                          kind="ExternalInput")
    kvbv = nc.dram_tensor("kvbv", [128, CKV // 128, HG * D_V], BF16,
                          kind="ExternalInput")
    ow = nc.dram_tensor("ow", [128, HG, HID], BF16, kind="ExternalInput")
    costl = nc.dram_tensor("costl", [64, 128], F32, kind="ExternalInput")
    sintl = nc.dram_tensor("sintl", [64, 128], F32, kind="ExternalInput")
    cos2t = nc.dram_tensor("cos2t", [128, S], BF16, kind="ExternalInput")
    sin2tg = nc.dram_tensor("sin2tg", [128, S], BF16, kind="ExternalInput")
    masks2 = nc.dram_tensor("masks2", [128, 2 * QW], BF16, kind="ExternalInput")
    sel = nc.dram_tensor("sel", [128, 4], BF16, kind="ExternalInput")
    onesb = nc.dram_tensor("onesb", [128, 128], F32R, kind="ExternalInput")
    out = nc.dram_tensor("out", [S, HID], BF16, kind="ExternalOutput")

    with tile.TileContext(nc) as tc, ExitStack() as top, \
            nc.allow_low_precision(reason="bf16 kernel, 2e-2 rel gate"):
        const = top.enter_context(tc.tile_pool(name="const", bufs=1))
        dram = top.enter_context(tc.tile_pool(name="dram", bufs=1, space="DRAM"))
        ps_pj = top.enter_context(tc.tile_pool(name="ps_pj", bufs=2, space="PSUM"))

        # ---- constants ----
        masks_sb = const.tile([128, 2 * QW], BF16, tag="masks")
        nc.sync.dma_start(masks_sb[:], masks2.ap())
        cos2t_sb = const.tile([128, S], BF16, tag="cos2t")
        sin2tg_sb = const.tile([128, S], BF16, tag="sin2tg")
        nc.sync.dma_start(cos2t_sb[:], cos2t.ap())
        nc.sync.dma_start(sin2tg_sb[:], sin2tg.ap())
        onesb_sb = const.tile([128, 128], F32R, tag="onesb")
        nc.sync.dma_start(onesb_sb[:], onesb.ap())
        sel_sb = const.tile([128, 4], BF16, tag="sel")
        nc.sync.dma_start(sel_sb[:], sel.ap())
        costl_sb = const.tile([64, 128], F32, tag="costl")
        sintl_sb = const.tile([64, 128], F32, tag="sintl")
        nc.sync.dma_start(costl_sb[:], costl.ap())
        nc.sync.dma_start(sintl_sb[:], sintl.ap())

        agi1 = dram.tile([130, S], BF16, tag="agi1")
        ago1 = dram.tile([130 * N_CORES, S], BF16, tag="ago1",
                         addr_space="Shared")
        agi2 = dram.tile([194, S], BF16, tag="agi2")
        ago2 = dram.tile([194 * N_CORES, S], BF16, tag="ago2",
                         addr_space="Shared")

        # ================= Phase A: col-sharded fused a-proj ================
        # Two passes: pass1 computes the ckv+kpe column slice and ships AG1
        # immediately; pass2 (q columns) overlaps the AG1 collective.
        with ExitStack() as pa:
            sba = pa.enter_context(tc.tile_pool(name="sba", bufs=1))
            psa = pa.enter_context(tc.tile_pool(name="psa", bufs=1, space="PSUM"))

            ident = sba.tile([128, 128], BF16, tag="ident")
            make_identity(nc, ident[:])
            # warm the PE clock (HAM releases the 4/8 throttle after ~3.4us
            # of sustained activity) while the input DMAs stream in
            junk = psa.tile([128, 128], F32, tag="pss")
            for _ in range(400):
                nc.tensor.matmul(junk[:], ident[:], ident[:],
                                 start=True, stop=True)

            xt = sba.tile([128, CC_A, S], BF16, tag="xt")
            xkpt = sba.tile([128, CC_A, 128], BF16, tag="xkpt")
            w1t = sba.tile([128, CC_A, 128], BF16, tag="w1t")
            w2t = sba.tile([128, CC_A, 128], BF16, tag="w2t")
            wkt = sba.tile([128, CC_A, 64], BF16, tag="wkt")
            nc.sync.dma_start(w2t[:], wag2.ap())
            nc.sync.dma_start(wkt[:], wakp.ap())
            nc.sync.dma_start(xkpt[:], xkp.ap())
            for qq in range(4):
                nc.sync.dma_start(xt[:, qq * 14:(qq + 1) * 14, :],
                                  xT.ap()[:, qq * 14:(qq + 1) * 14, :])
            nc.sync.dma_start(w1t[:], wag1.ap())

            # ---- pass 1: ckv + kpe columns ----
            psa2 = [psa.tile([128, 512], F32, tag=f"psa2{st}", name=f"psa2{st}")
                    for st in range(2)]
            pskp = psa.tile([64, 128], F32, tag="pskp")
            for cc in range(CC_A):
                st_, sp_ = (cc == 0), (cc == CC_A - 1)
                for st in range(2):
                    nc.tensor.matmul(psa2[st][:], w2t[:, cc, :],
                                     xt[:, cc, st * 512:(st + 1) * 512],
                                     start=st_, stop=sp_)
                nc.tensor.matmul(pskp[:], wkt[:, cc, :], xkpt[:, cc, :],
                                 start=st_, stop=sp_)

            a2b = sba.tile([128, S], BF16, tag="a2b")
            for st in range(2):
                nc.vector.tensor_copy(a2b[:, st * 512:(st + 1) * 512], psa2[st][:])
            # k_pe rope on own 128-seq slice (transposed [64, 128] layout)
            kpraw = sba.tile([64, 128], F32, tag="kpraw")
            nc.vector.tensor_copy(kpraw[:], pskp[:])
            kv1 = sba.tile([64, 128], F32, tag="kv1")
            kvs = sba.tile([64, 128], F32, tag="kvs")
            nc.vector.tensor_mul(kv1[:], kpraw[:], costl_sb[:])
            nc.vector.tensor_copy(kvs[0:32, :], kpraw[32:64, :])
            nc.vector.tensor_copy(kvs[32:64, :], kpraw[0:32, :])
            nc.vector.tensor_mul(kvs[:], kvs[:], sintl_sb[:])
            kpb = sba.tile([64, 128], BF16, tag="kpb")
            nc.vector.tensor_add(kpb[:], kv1[:], kvs[:])
            # ckv partial mean-square row (sel col3 = 1/CKV on rows 64:128)
            sq2 = sba.tile([128, S], BF16, tag="sq2")
            nc.vector.tensor_mul(sq2[:], a2b[:], a2b[:])
            ssb1 = sba.tile([1, S], BF16, tag="ssb1")
            for st in range(2):
                pss = psa.tile([1, 512], F32, tag="pss")
                nc.tensor.matmul(pss[:], sel_sb[:, 3:4],
                                 sq2[:, st * 512:(st + 1) * 512],
                                 start=True, stop=True)
                nc.vector.tensor_copy(ssb1[:, st * 512:(st + 1) * 512], pss[:])
            nc.sync.dma_start(agi1[0:64, :], a2b[64:128, :])
            nc.sync.dma_start(agi1[64:128, 0:128], kpb[:])
            nc.sync.dma_start(agi1[128:129, :], ssb1[:])

            nc.gpsimd.collective_compute(
                "AllGather", OP.bypass,
                replica_groups=[list(range(N_CORES))],
                ins=[agi1.opt()], outs=[ago1.opt()],
            )

            # ---- pass 2: q columns (overlaps AG1) ----
            psa1 = [psa.tile([128, 512], F32, tag=f"psa2{st}", name=f"psa1{st}")
                    for st in range(2)]
            for cc in range(CC_A):
                st_, sp_ = (cc == 0), (cc == CC_A - 1)
                for st in range(2):
                    nc.tensor.matmul(psa1[st][:], w1t[:, cc, :],
                                     xt[:, cc, st * 512:(st + 1) * 512],
                                     start=st_, stop=sp_)
            a1b = sba.tile([128, S], BF16, tag="a1b")
            for st in range(2):
                nc.vector.tensor_copy(a1b[:, st * 512:(st + 1) * 512], psa1[st][:])
            sq1 = sba.tile([128, S], BF16, tag="sq1")
            nc.vector.tensor_mul(sq1[:], a1b[:], a1b[:])
            ssb2 = sba.tile([1, S], BF16, tag="ssb2")
            for st in range(2):
                pss = psa.tile([1, 512], F32, tag="pss")
                nc.tensor.matmul(pss[:], sel_sb[:, 0:1],
                                 sq1[:, st * 512:(st + 1) * 512],
                                 start=True, stop=False)
                nc.tensor.matmul(pss[:], sel_sb[:, 2:3],
                                 sq2[:, st * 512:(st + 1) * 512],
                                 start=False, stop=True)
                nc.vector.tensor_copy(ssb2[:, st * 512:(st + 1) * 512], pss[:])
            nc.sync.dma_start(agi2[0:128, :], a1b[:])
            nc.sync.dma_start(agi2[128:192, :], a2b[0:64, :])
            nc.sync.dma_start(agi2[192:193, :], ssb2[:])

        nc.gpsimd.collective_compute(
            "AllGather", OP.bypass,
            replica_groups=[list(range(N_CORES))],
            ins=[agi2.opt()], outs=[ago2.opt()],
        )

        # ================= Phase B =========================================
        with ExitStack() as pb:
            sbo = pb.enter_context(tc.tile_pool(name="sbo", bufs=1))
            outT = sbo.tile([128, HG, S], BF16, tag="outT")
            pbi = pb.enter_context(ExitStack())
            sbq = pbi.enter_context(tc.tile_pool(name="sbq", bufs=1))
            sbvv = pbi.enter_context(tc.tile_pool(name="sbvv", bufs=1))
            sbkw = pbi.enter_context(tc.tile_pool(name="sbkw", bufs=1))
            sbn = pbi.enter_context(tc.tile_pool(name="sbn", bufs=1))
            sbw = pbi.enter_context(tc.tile_pool(name="sbw", bufs=2))
            sbg = pbi.enter_context(tc.tile_pool(name="sbg", bufs=2))
            sbgr = pbi.enter_context(tc.tile_pool(name="sbgr", bufs=1))
            sbs = pbi.enter_context(tc.tile_pool(name="sbs", bufs=2))
            ps_sc = pbi.enter_context(tc.tile_pool(name="ps_sc", bufs=3, space="PSUM"))
            ps_o = pbi.enter_context(tc.tile_pool(name="ps_o", bufs=2, space="PSUM"))
            ps_sb = pbi.enter_context(tc.tile_pool(name="ps_sb", bufs=1, space="PSUM"))

            # --- v-proj weights (no dep on collectives: prefetch early) ---
            kvbvt = sbkw.tile([128, CKV // 128, HG * D_V], BF16, tag="kvbvt")
            nc.sync.dma_start(kvbvt[:], kvbv.ap())

            ago1v = ago1.rearrange("(c r) s -> r c s", r=130)
            ago1p = ago1.rearrange("(cp two r) s -> two r cp s", two=2, r=130)
            ago2v = ago2.rearrange("(c r) s -> r c s", r=194)
            ago2p = ago2.rearrange("(cp two r) s -> two r cp s", two=2, r=194)

            # --- AG1 unpack: ckv chunks (raw, unnormalized), kpe2, ss row ---
            ckv = sbq.tile([128, CKV // 128, S], BF16, tag="ckv")
            nc.sync.dma_start(ckv[0:64, :, :], ago1p[0, 0:64, :, :])
            nc.sync.dma_start(ckv[64:128, :, :], ago1p[1, 0:64, :, :])
            kpe2 = sbq.tile([128, S], BF16, tag="kpe2")
            nc.sync.dma_start(
                kpe2[0:64, :].rearrange("r (c s) -> r c s", c=8),
                ago1v[64:128, :, 0:128])
            nc.vector.tensor_copy(kpe2[64:128, :], kpe2[0:64, :])
            ssk8 = sbn.tile([8, S], BF16, tag="ssk8")
            nc.sync.dma_start(ssk8[:], ago1v[128, :, :])

            # --- fk = rsqrt(mean+eps): broadcast row and per-seq column ---
            # (runs off the critical path: kTn/v are computed from the RAW
            # ckv; normalization is folded into their psum->sbuf copies)
            prk = sbn.tile([8, S], F32, tag="prk")
            nc.gpsimd.partition_all_reduce(prk[:], ssk8[:], channels=8,
                                           reduce_op=RED.add)
            fkrow = sbn.tile([1, S], F32, tag="rows")
            nc.vector.tensor_scalar_add(fkrow[:], prk[0:1, :], EPS)
            nc.vector.reciprocal(fkrow[:], fkrow[:])
            nc.scalar.activation(fkrow[:], fkrow[:], AF.Sqrt)
            fkb = sbn.tile([128, S], F32, tag="fkb")
            nc.gpsimd.partition_broadcast(fkb[:], fkrow[:])
            # row -> per-seq column layout, bounced through DRAM (an SBUF
            # free-dim range cannot be reinterpreted as partitions directly)
            fkstg = dram.tile([1, S], F32, tag="fkstg")
            nc.sync.dma_start(fkstg[:], fkrow[:])
            fkcol = sbn.tile([128, 8], F32, tag="fkcol")
            nc.sync.dma_start(
                fkcol[:], fkstg.rearrange("o (c p) -> (o p) c", p=128))

            # --- v-proj for ALL heads from raw ckv (covers AG2 latency);
            # fk applied afterwards per seq-chunk partition-scalar ---
            vv = sbvv.tile([128, 8, HG * D_V], BF16, tag="vv")
            for g in range(N_GROUPS):
                for sc in range(8):
                    pj = ps_pj.tile([128, 512], F32, tag="pj")
                    for ccc in range(CKV // 128):
                        nc.tensor.matmul(
                            pj[:, 0:256],
                            ckv[:, ccc, sc * 128:(sc + 1) * 128],
                            kvbvt[:, ccc, g * 256:(g + 1) * 256],
                            start=(ccc == 0), stop=(ccc == CKV // 128 - 1))
                    nc.vector.tensor_copy(vv[:, sc, g * 256:(g + 1) * 256],
                                          pj[:, 0:256])
            for sc in range(8):
                nc.vector.tensor_scalar_mul(vv[:, sc, :], vv[:, sc, :],
                                            fkcol[:, sc:sc + 1])

            # --- AG2 unpack: raw q chunks + fq (folded into q-proj copies) ---
            qch = sbq.tile([128, CQ // 128, S], BF16, tag="qch")
            nc.sync.dma_start(qch[:, 0:8, :], ago2v[0:128, :, :])
            nc.sync.dma_start(qch[0:64, 8:12, :], ago2p[0, 128:192, :, :])
            nc.sync.dma_start(qch[64:128, 8:12, :], ago2p[1, 128:192, :, :])
            ssq8 = sbn.tile([8, S], BF16, tag="ssk8")
            nc.sync.dma_start(ssq8[:], ago2v[192, :, :])
            prq = sbn.tile([8, S], F32, tag="prk")
            nc.gpsimd.partition_all_reduce(prq[:], ssq8[:], channels=8,
                                           reduce_op=RED.add)
            fqrow = sbn.tile([1, S], F32, tag="rows")
            nc.vector.tensor_scalar_add(fqrow[:], prq[0:1, :], EPS)
            nc.vector.reciprocal(fqrow[:], fqrow[:])
            nc.scalar.activation(fqrow[:], fqrow[:], AF.Sqrt)
            fqb = sbn.tile([128, S], F32, tag="fqb")
            nc.gpsimd.partition_broadcast(fqb[:], fqrow[:])

            # softmax denominators for all (head, q-tile); one batched
            # reciprocal at the end (avoids slow [1,N] reciprocals)
            sums_all = sbn.tile([HG, S], F32, tag="sums_all")
            recip_all = sbn.tile([HG, S], F32, tag="recip_all")

            # --- per head-pair group: k/q projections + rope + attention ---
            for g in range(N_GROUPS):
                qbnt = sbw.tile([128, CQ // 128, 256], BF16, tag="qbnt")
                qbpt = sbw.tile([128, CQ // 128, 128], BF16, tag="qbpt")
                kvbkg = sbw.tile([128, CKV // 128, 256], BF16, tag="kvbkg")
                nc.sync.dma_start(qbnt[:], qbn.ap()[:, :, g * 256:(g + 1) * 256])
                nc.sync.dma_start(qbpt[:], qbp.ap()[:, :, g * 128:(g + 1) * 128])
                nc.sync.dma_start(kvbkg[:], kvbk.ap()[:, :, g * 256:(g + 1) * 256])

                kTn = sbg.tile([128, 2, S], BF16, tag="kTn")
                for i in range(2):
                    for st in range(2):
                        pj = ps_pj.tile([128, 512], F32, tag="pj")
                        for ccc in range(CKV // 128):
                            nc.tensor.matmul(
                                pj[:], kvbkg[:, ccc, i * 128:(i + 1) * 128],
                                ckv[:, ccc, st * 512:(st + 1) * 512],
                                start=(ccc == 0), stop=(ccc == CKV // 128 - 1))
                        nc.vector.tensor_mul(kTn[:, i, st * 512:(st + 1) * 512],
                                             pj[:],
                                             fkb[:, st * 512:(st + 1) * 512])
                qTn2 = sbg.tile([128, 2, S], BF16, tag="qTn2")
                for i in range(2):
                    for st in range(2):
                        pj = ps_pj.tile([128, 512], F32, tag="pj")
                        for ccc in range(CQ // 128):
                            nc.tensor.matmul(
                                pj[:], qbnt[:, ccc, i * 128:(i + 1) * 128],
                                qch[:, ccc, st * 512:(st + 1) * 512],
                                start=(ccc == 0), stop=(ccc == CQ // 128 - 1))
                        nc.vector.tensor_mul(qTn2[:, i, st * 512:(st + 1) * 512],
                                             pj[:],
                                             fqb[:, st * 512:(st + 1) * 512])
                qTp = sbg.tile([128, S], BF16, tag="qTp")
                qpr = sbgr.tile([128, S], BF16, tag="qpr")
                for st in range(2):
                    pj = ps_pj.tile([128, 512], F32, tag="pj")
                    for ccc in range(CQ // 128):
                        nc.tensor.matmul(
                            pj[:], qbpt[:, ccc, :],
                            qch[:, ccc, st * 512:(st + 1) * 512],
                            start=(ccc == 0), stop=(ccc == CQ // 128 - 1))
                    nc.vector.tensor_mul(qpr[:, st * 512:(st + 1) * 512], pj[:],
                                         fqb[:, st * 512:(st + 1) * 512])
                rs = sbgr.tile([128, S], BF16, tag="rs")
                nc.vector.tensor_mul(qTp[:], qpr[:], cos2t_sb[:])
                for b in range(4):
                    r0 = b * 32
                    r1 = r0 + 32 if b % 2 == 0 else r0 - 32
                    nc.vector.tensor_copy(rs[r0:r0 + 32, :], qpr[r1:r1 + 32, :])
                nc.vector.tensor_mul(rs[:], rs[:], sin2tg_sb[:])
                nc.vector.tensor_add(qTp[:], qTp[:], rs[:])

                for i in range(2):
                    h = 2 * g + i
                    b = i * 64
                    for qt in range(QT):
                        po = ps_o.tile([128, QW], F32, tag="po")
                        sums = sbs.tile([128, 2 * QW], F32R, tag="sums")
                        for j in range(qt + 1):
                            ps = ps_sc.tile([128, 2 * QW], F32, tag="ps")
                            for half in range(2):
                                kc = 2 * j + half
                                hs = slice(half * QW, (half + 1) * QW)
                                nc.tensor.matmul(
                                    ps[:, hs],
                                    kTn[:, i, kc * 128:(kc + 1) * 128],
                                    qTn2[:, i, qt * QW:(qt + 1) * QW],
                                    start=True, stop=False)
                                nc.tensor.matmul(
                                    ps[:, hs],
                                    kpe2[b:b + 64, kc * 128:(kc + 1) * 128],
                                    qTp[b:b + 64, qt * QW:(qt + 1) * QW],
                                    start=False, stop=True)
                            pt = sbs.tile([128, 2 * QW], BF16, tag="pt")
                            nc.scalar.activation(pt[:], ps[:], AF.Exp, scale=SCALE)
                            if j == qt:
                                nc.vector.tensor_mul(pt[:], pt[:], masks_sb[:])
                            if j == 0:
                                nc.vector.tensor_copy(sums[:], pt[:])
                            else:
                                nc.vector.tensor_add(sums[:], sums[:], pt[:])
                            nc.tensor.matmul(
                                po[:], vv[:, 2 * j, h * 128:(h + 1) * 128],
                                pt[:, 0:QW], start=(j == 0), stop=False)
                            nc.tensor.matmul(
                                po[:], vv[:, 2 * j + 1, h * 128:(h + 1) * 128],
                                pt[:, QW:2 * QW], start=False, stop=(j == qt))
                        psb = ps_sb.tile([1, QW], F32, tag="psb")
                        nc.tensor.matmul(psb[:], onesb_sb[:, 0:1], sums[:, 0:QW],
                                         start=True, stop=False)
                        nc.tensor.matmul(psb[:], onesb_sb[:, 0:1], sums[:, QW:],
                                         start=False, stop=True)
                        srow = sbs.tile([1, QW], F32, tag="srow")
                        nc.scalar.copy(srow[:], psb[:])
                        nc.sync.dma_start(
                            sums_all[h:h + 1, qt * QW:(qt + 1) * QW], srow[:])
                        nc.vector.tensor_copy(
                            outT[:, h, qt * QW:(qt + 1) * QW], po[:])

            # normalize all heads' outputs: one batched reciprocal, then
            # per-head broadcast + multiply
            nc.vector.reciprocal(recip_all[:], sums_all[:])
            for h in range(HG):
                rrow = sbn.tile([1, S], F32, tag="rows")
                nc.sync.dma_start(rrow[:], recip_all[h:h + 1, :])
                rb = sbn.tile([128, S], F32, tag="bcast")
                nc.gpsimd.partition_broadcast(rb[:], rrow[:])
                nc.vector.tensor_mul(outT[:, h, :], outT[:, h, :], rb[:])

            pbi.close()  # free all B scratch before phase C pools

            # ================= Phase C: output projection ==================
            with ExitStack() as pc:
                sbow = pc.enter_context(tc.tile_pool(name="sbow", bufs=3))
                sbos = pc.enter_context(tc.tile_pool(name="sbos", bufs=2))
                outv = out.ap().rearrange("(st p) n -> p st n", p=128)
                for nt in range(HID // 512):
                    owt = sbow.tile([128, HG, 512], BF16, tag="ow")
                    nc.sync.dma_start(owt[:], ow.ap()[:, :, nt * 512:(nt + 1) * 512])
                    osb = sbos.tile([128, 8, 512], BF16, tag="osb")
                    for st in range(8):
                        pj = ps_pj.tile([128, 512], F32, tag="pj")
                        for hc in range(HG):
                            nc.tensor.matmul(
                                pj[:], outT[:, hc, st * 128:(st + 1) * 128],
                                owt[:, hc, :], start=(hc == 0), stop=(hc == HG - 1))
                        nc.vector.tensor_copy(osb[:, st, :], pj[:])
                    nc.sync.dma_start(outv[:, :, nt * 512:(nt + 1) * 512], osb[:])

    nc.compile()
    return nc


def _tile_rows(a, p=128):
    """[R, N] -> [p, R//p, N] with row r = (cc*p + part)."""
    r, n = a.shape
    return np.ascontiguousarray(a.reshape(r // p, p, n).transpose(1, 0, 2))


def _host_inputs(hidden_states, position_ids, q_a_weight, q_a_layernorm_weight,
                 q_b_weight, kv_a_weight, kv_a_layernorm_weight, kv_b_weight,
                 o_weight):
    bf = ml_dtypes.bfloat16
    x = np.asarray(hidden_states, np.float32).reshape(S, HID)
    pos = np.asarray(position_ids, np.float64).reshape(S)
    q_a_w = np.asarray(q_a_weight, np.float32)
    q_ln = np.asarray(q_a_layernorm_weight, np.float32)
    q_b_w = np.asarray(q_b_weight, np.float32)
    kv_a_w = np.asarray(kv_a_weight, np.float32)
    kv_ln = np.asarray(kv_a_layernorm_weight, np.float32)
    kv_b_w = np.asarray(kv_b_weight, np.float32)
    o_w = np.asarray(o_weight, np.float32)

    xT = np.ascontiguousarray(x.T).astype(bf)                 # [HID, S]
    xT_t = _tile_rows(xT)                                     # [128, 56, S]

    qb = (q_ln[:, None] * q_b_w).reshape(CQ, H, D_Q)
    kvb = (kv_ln[:, None] * kv_b_w).reshape(CKV, H, D_NOPE + D_V)

    inv_freq = 1.0 / (10000.0 ** (np.arange(0, D_ROPE, 2, dtype=np.float64) / D_ROPE))
    freqs = pos[:, None] * inv_freq[None, :]
    emb = np.concatenate([freqs, freqs], axis=-1)
    cos = np.cos(emb).astype(np.float32)
    sin = np.sin(emb).astype(np.float32)
    sin_sg = np.concatenate([-sin[:, :32], sin[:, 32:]], axis=1)
    cosT = np.ascontiguousarray(cos.T)                        # [64, S]
    sinT_sg = np.ascontiguousarray(sin_sg.T)
    cos2t = np.concatenate([cosT, cosT], axis=0).astype(bf)
    sin2tg = np.concatenate([sinT_sg, sinT_sg], axis=0).astype(bf)

    i = np.arange(128)[:, None]
    j = np.arange(QW)[None, :]
    m0 = (i <= j).astype(np.float32)
    m1 = ((i + 128) <= j).astype(np.float32)
    masks2 = np.concatenate([m0, m1], axis=1).astype(bf)

    # mean-square selectors, pre-scaled by 1/CQ (q cols) and 1/CKV (ckv)
    sel = np.zeros((128, 4), np.float32)
    sel[:, 0] = 1.0 / CQ
    sel[0:64, 2] = 1.0 / CQ
    sel[64:128, 3] = 1.0 / CKV
    sel = sel.astype(bf)

    onesb = np.ones((128, 128), np.float32)

    qbn_f = qb[:, :, :D_NOPE]
    qbp_f = qb[:, :, D_NOPE:]
    kvbk_f = kvb[:, :, :D_NOPE]
    kvbv_f = kvb[:, :, D_NOPE:]

    in_maps = []
    for c in range(N_CORES):
        hs = slice(c * HG, (c + 1) * HG)
        wag1 = q_a_w[:, 128 * c:128 * (c + 1)]
        wag2 = np.concatenate([
            q_a_w[:, 1024 + 64 * c:1024 + 64 * (c + 1)],
            kv_a_w[:, 64 * c:64 * (c + 1)]], axis=1)
        ow_c = o_w[c * HG * D_V:(c + 1) * HG * D_V, :].astype(bf)  # [2048, HID]
        in_maps.append({
            "xT": xT_t,
            "xkp": _tile_rows(np.ascontiguousarray(
                xT[:, 128 * c:128 * (c + 1)])),
            "wag1": _tile_rows(np.ascontiguousarray(wag1).astype(bf)),
            "wag2": _tile_rows(np.ascontiguousarray(wag2).astype(bf)),
            "wakp": _tile_rows(np.ascontiguousarray(kv_a_w[:, CKV:]).astype(bf)),
            "qbn": _tile_rows(np.ascontiguousarray(
                qbn_f[:, hs, :].reshape(CQ, HG * D_NOPE)).astype(bf)),
            "qbp": _tile_rows(np.ascontiguousarray(
                qbp_f[:, hs, :].reshape(CQ, HG * D_ROPE)).astype(bf)),
            "kvbk": _tile_rows(np.ascontiguousarray(
                kvbk_f[:, hs, :].reshape(CKV, HG * D_NOPE)).astype(bf)),
            "kvbv": _tile_rows(np.ascontiguousarray(
                kvbv_f[:, hs, :].reshape(CKV, HG * D_V)).astype(bf)),
            "ow": _tile_rows(ow_c),                           # [128, 16, HID]
            "costl": np.ascontiguousarray(cosT[:, 128 * c:128 * (c + 1)]),
            "sintl": np.ascontiguousarray(sinT_sg[:, 128 * c:128 * (c + 1)]),
            "cos2t": cos2t,
            "sin2tg": sin2tg,
            "masks2": masks2,
            "sel": sel,
            "onesb": onesb,
        })
    return in_maps


def kernel(**inputs):
    global LAST_EXEC_NS, LAST_RES
    trace = bool(inputs.pop("_trace", False))
    in_maps = _host_inputs(**inputs)
    if "nc" not in _CACHE:
        _CACHE["nc"] = _build_nc()
    nc = _CACHE["nc"]
    res = bass_utils.run_bass_kernel_spmd(
        nc, in_maps, core_ids=list(range(N_CORES)), trace=trace)
    LAST_EXEC_NS = res.exec_time_ns
    LAST_RES = res
    total = np.zeros((S, HID), np.float64)
    for c in range(N_CORES):
        total += res.results[c]["out"].astype(np.float64)
    return total.astype(np.float32).reshape(1, 1, S, HID)
